# revision 2
# baseline (speedup 1.0000x reference)
"""Trainium2 Bass kernel v2 for masked dual-softmax attention.

Reference (per batch b, head h; dh=16, H=8, N=1024, D=128):
  q = query @ Wq + bq ; k = key @ Wk + bk ; v = value @ Wv + bv
  S = q_h k_h^T / sqrt(dh)
  attn = 0.5*(softmax(S) + softmax(S masked by adj))
  out = concat_h(attn @ v_h) @ Wo + bo

Sharding: data-parallel over batch, one batch element per core (8 cores).

v2 device algorithm (S^T layout: m=key idx on partitions, n=query idx free):
  - projections as v1 (head-permuted weight tiles, quadrant packing);
    q/k bias adds fused into the PSUM->SBUF copy (qa/ka on DVE, qb/kb Pool)
  - per (nh, g2) group: 8 m-chunks of S^T [128,1024] (2 heads x 512 n),
    exp -> eg (ACT exp, or bitcast fast-exp on Pool/DVE), em = eg*mask (DVE)
  - attn@V transposed: og[n, 17] = sum_mc eg_chunk^T @ [1|v_h]; the 17-wide
    moving operand costs PE 17 rows instead of 512. 16 sequential
    accumulators per group (PSUM zero-region rule forbids interleaving
    accumulation groups within one bank)
  - normalize on DVE (strided sum gather, reciprocal_approx_fast, one fused
    multiply) into attn_sb [n, k] tiles
  - [n,k] -> [k,n] via PE transposes (identity matmul) + DVE/Pool copies
  - output projection: out[n,d] = catT_g^T (0.5 Wo) + catT_m^T (0.5 Wo) + bo;
    result DMA'd to DRAM directly from PSUM
  - PE emission interleaves group gi's score matmuls with group gi-1's og
    accumulators; cross-half tails are emitted 2 groups late so the PE
    in-order stream never head-of-line blocks the ACT exp chain.
"""

import sys

if "/opt/trn_rl_repo" not in sys.path:
    sys.path.insert(0, "/opt/trn_rl_repo")

import numpy as np
import ml_dtypes
from contextlib import ExitStack

B, N, D, H, DH = 8, 1024, 128, 8, 16
NCORES = 8
P = 128
NH = 2          # n halves of 512
NHF = N // NH   # 512
MC = 8          # m chunks of 128
G2 = 4          # head groups of 2
NC4 = 4         # n chunks of 128 per half
NG = NH * G2    # 8 groups

_BF16 = ml_dtypes.bfloat16
_CACHE = {}

# exp schedule: "act" (native exp) or "dve" (bitcast fast-exp; DVE only --
# GPSIMD cannot read PSUM). mask schedule: "dve" or "pool".
EXP_SCHEDULE = {(g, mc): "act" for g in range(NG) for mc in range(MC)}
for _g in range(NG):
    EXP_SCHEDULE[(_g, 0)] = "dve"

# fast-exp: exp(x) ~= bf16_bits( uint16( x*A16 + B16 ) )
# (top 16 bits of the classic int32 bitcast trick; uint16 write rounds)
FE_A16 = float(2 ** 23 / np.log(2.0) / 65536.0)
FE_B16 = float((127 * 2 ** 23 - 366000.0) / 65536.0)


def _build_nc():
    import concourse.tile as tile
    import concourse.mybir as mybir
    from concourse import bacc

    bf16 = mybir.dt.bfloat16
    f32 = mybir.dt.float32
    f32r = mybir.dt.float32r
    u16 = mybir.dt.uint16
    Exp = mybir.ActivationFunctionType.Exp
    Alu = mybir.AluOpType

    nc = bacc.Bacc("TRN2", target_bir_lowering=False, debug=False,
                   num_devices=NCORES)

    # ---- DRAM I/O -------------------------------------------------------
    # c32: wqa|wqb|wka|wkb (4x128 cols) + bqa|bqb|bka|bkb (4 cols)
    c32_d = nc.dram_tensor("c32", [P, 4 * P + 4], f32r, kind="ExternalInput")
    # cbf: wv|wog|identity (3x128 cols)
    cbf_d = nc.dram_tensor("cbf", [P, 3 * P], bf16, kind="ExternalInput")
    # br: bvr|bor rows
    br_d = nc.dram_tensor("br", [1, 2 * P], bf16, kind="ExternalInput")
    xq_d = nc.dram_tensor("xqT", [P, N], f32r, kind="ExternalInput")
    xk_d = nc.dram_tensor("xkT", [P, N], f32r, kind="ExternalInput")
    xv_d = nc.dram_tensor("xvT", [P, N], bf16, kind="ExternalInput")
    mask_d = nc.dram_tensor("maskL", [P, MC * NH * NHF], bf16, kind="ExternalInput")
    out_d = nc.dram_tensor("out", [N, D], f32, kind="ExternalOutput")

    with tile.TileContext(nc) as tc, ExitStack() as ctx:
        const = ctx.enter_context(tc.tile_pool(name="const", bufs=1))
        xpool = ctx.enter_context(tc.tile_pool(name="x", bufs=1))
        qkpool = ctx.enter_context(tc.tile_pool(name="qk", bufs=1))
        egp = ctx.enter_context(tc.tile_pool(name="eg", bufs=3))
        emp = ctx.enter_context(tc.tile_pool(name="em", bufs=3))
        recp = ctx.enter_context(tc.tile_pool(name="rec", bufs=2))
        atp = ctx.enter_context(tc.tile_pool(name="attn", bufs=2))
        ctp = ctx.enter_context(tc.tile_pool(name="catT", bufs=2))
        osb = ctx.enter_context(tc.tile_pool(name="osb", bufs=2))
        # PSUM: s4 2 banks x2, og 1 bank x2, outp 1 bank, fexp 1 bank = 8
        s4p = ctx.enter_context(tc.tile_pool(name="s4", bufs=2, space="PSUM"))
        ogp = ctx.enter_context(tc.tile_pool(name="og", bufs=2, space="PSUM"))
        outp = ctx.enter_context(tc.tile_pool(name="outp", bufs=1, space="PSUM"))
        fxp = ctx.enter_context(tc.tile_pool(name="fxp", bufs=1, space="PSUM"))

        # ---- constants / inputs (order = DMA issue order) ----------------
        c32 = const.tile([P, 4 * P + 4], f32r, tag="c32")
        nc.sync.dma_start(c32[:], c32_d.ap())
        xq = xpool.tile([P, N], f32r, tag="xq")
        xk = xpool.tile([P, N], f32r, tag="xk")
        xv = xpool.tile([P, N], bf16, tag="xv")
        nc.sync.dma_start(xq[:], xq_d.ap())
        nc.sync.dma_start(xk[:], xk_d.ap())
        nc.sync.dma_start(xv[:], xv_d.ap())
        cbf = const.tile([P, 3 * P], bf16, tag="cbf")
        nc.sync.dma_start(cbf[:], cbf_d.ap())
        br = const.tile([1, 2 * P], bf16, tag="br")
        nc.sync.dma_start(br[:], br_d.ap())
        mask_sb = const.tile([P, MC, NH, NHF], bf16, tag="mask")
        mask_dr = mask_d.ap().rearrange("p (a b f) -> p a b f", a=MC, b=NH)
        for _mc in range(MC):
            nc.sync.dma_start(mask_sb[:, _mc, :, :], mask_dr[:, _mc, :, :])

        wqa, wqb = c32[:, 0:P], c32[:, P:2 * P]
        wka, wkb = c32[:, 2 * P:3 * P], c32[:, 3 * P:4 * P]
        bias_col = {"qa": c32[:, 4 * P + 0:4 * P + 1].bitcast(f32),
                    "qb": c32[:, 4 * P + 1:4 * P + 2].bitcast(f32),
                    "ka": c32[:, 4 * P + 2:4 * P + 3].bitcast(f32),
                    "kb": c32[:, 4 * P + 3:4 * P + 4].bitcast(f32)}
        wv, wog, ident = cbf[:, 0:P], cbf[:, P:2 * P], cbf[:, 2 * P:3 * P]
        bvr, bor = br[:, 0:P], br[:, P:2 * P]

        ones1 = const.tile([1, P], bf16, tag="ones1")
        nc.vector.memset(ones1[:], 1.0)

        # ---- PE warm-up: ramp the p-state while input DMAs land ----------
        warm = ogp.tile([P, 512], f32, tag="og", name="warm")
        for _w in range(28):
            nc.tensor.matmul(warm[:, 0:P], ones1[:], ones1[:],
                             start=True, stop=True)

        # ---- q/k projections (head-permuted quadrant packing) ------------
        # qa/ka use the two 2-bank s4 slots (freed ASAP for the score
        # chunks); qb/kb go through the 1-bank fxp / og slots in halves so
        # they never block the s4 rotation.
        qk_tiles = {}
        for name in ("qa", "ka", "qb", "kb"):
            qk_tiles[name] = qkpool.tile([P, N], f32r, tag=name, name=name)
        # qa: n-half 1 through the fxp bank, copied on ACT (fills early ACT
        # idle; covers all nh=0 score chunks); n-half 2 via og bank on DVE
        qah1 = fxp.tile([P, NHF], f32, tag="sf", name="qah1")
        nc.tensor.matmul(qah1[:], wqa, xq[:, 0:NHF], start=True, stop=True)
        nc.scalar.add(qk_tiles["qa"][:, 0:NHF], qah1[:], bias_col["qa"])
        qah2 = ogp.tile([P, 512], f32, tag="og", name="qah2")
        nc.tensor.matmul(qah2[:], wqa, xq[:, NHF:N], start=True, stop=True)
        nc.vector.tensor_scalar_add(qk_tiles["qa"][:, NHF:N], qah2[:],
                                    bias_col["qa"])
        # ka: s4-pool psum; m-cols 0:256 copied first (unblocks s4(0,1))
        ps_ka = s4p.tile([P, N], f32, tag="s4", name="proj_ka")
        for s in range(NH):
            nc.tensor.matmul(ps_ka[:, s * NHF:(s + 1) * NHF], wka,
                             xk[:, s * NHF:(s + 1) * NHF],
                             start=True, stop=True)
        nc.vector.tensor_scalar_add(qk_tiles["ka"][:, 0:256],
                                    ps_ka[:, 0:256], bias_col["ka"])
        nc.vector.tensor_scalar_add(qk_tiles["ka"][:, 256:N],
                                    ps_ka[:, 256:N], bias_col["ka"])
        def emit_b_projections():
            # qb: half 1 in the fxp bank, half 2 in an og bank
            qbh1 = fxp.tile([P, NHF], f32, tag="sf", name="qbh1")
            nc.tensor.matmul(qbh1[:], wqb, xq[:, 0:NHF], start=True, stop=True)
            nc.vector.tensor_scalar_add(qk_tiles["qb"][:, 0:NHF], qbh1[:],
                                        bias_col["qb"])
            qbh2 = ogp.tile([P, 512], f32, tag="og", name="qbh2")
            nc.tensor.matmul(qbh2[:], wqb, xq[:, NHF:N], start=True, stop=True)
            nc.vector.tensor_scalar_add(qk_tiles["qb"][:, NHF:N], qbh2[:],
                                        bias_col["qb"])
            # kb: two og-bank halves
            kbh1 = ogp.tile([P, 512], f32, tag="og", name="kbh1")
            nc.tensor.matmul(kbh1[:], wkb, xk[:, 0:NHF], start=True, stop=True)
            nc.vector.tensor_scalar_add(qk_tiles["kb"][:, 0:NHF], kbh1[:],
                                        bias_col["kb"])
            kbh2 = ogp.tile([P, 512], f32, tag="og", name="kbh2")
            nc.tensor.matmul(kbh2[:], wkb, xk[:, NHF:N], start=True, stop=True)
            nc.vector.tensor_scalar_add(qk_tiles["kb"][:, NHF:N], kbh2[:],
                                        bias_col["kb"])

        # ---- v projection: natural layout [m, dim], augmented [1|v] ------
        # vaug[p, mc, h, 0] = 1 ; vaug[p, mc, h, 1+c] = V[mc*128+p, 16h+c]
        vaug = qkpool.tile([P, MC, H, 17], bf16, tag="vaug")
        nc.vector.memset(vaug[:, :, :, 0], 1.0)

        vaug_copies = []

        def emit_vp():
            for half in range(2):
                vp = outp.tile([P, 4, P], f32, tag="outp", name=f"vps{half}")
                for mc4 in range(4):
                    mc = half * 4 + mc4
                    sl = vp[:, mc4, :]
                    nc.tensor.matmul(sl, ones1[:], bvr,
                                     start=True, stop=False)
                    nc.tensor.matmul(sl, xv[:, mc * P:(mc + 1) * P], wv,
                                     start=False, stop=True)
                    vaug_copies.append((mc, sl))

        # ---- main loop --------------------------------------------------
        groups = [(nh, g2) for nh in range(NH) for g2 in range(G2)]
        state = {}

        def emit_og_accs(gi, accs, og=None, wbase=0):
            """Emit the given accumulator indices (0..15) of group gi."""
            nh, g2 = groups[gi]
            st = state[gi]
            og = og if og is not None else st["og"]
            for acc in accs:
                nc4, r = divmod(acc, 4)
                a, i = divmod(r, 2)
                tiles = st["egs"] if a == 0 else st["ems"]
                w = (acc - wbase) * 17
                for mc in range(MC):
                    nc.tensor.matmul(
                        og[:, w:w + 17],
                        tiles[mc][:, i * NHF + nc4 * P:i * NHF + (nc4 + 1) * P],
                        vaug[:, mc, 2 * g2 + i, :],
                        start=(mc == 0), stop=(mc == MC - 1))

        def emit_norm(gi, og=None, nc0=0, ncn=NC4, suffix=""):
            """Normalize accumulators for nc4 range [nc0, nc0+ncn) held in
            `og` (windows start at (nc4-nc0)*68 cols)."""
            nh, g2 = groups[gi]
            st = state[gi]
            og = og if og is not None else st["og"]
            nw = ncn * 4
            ogv = og[:, 0:17 * nw].rearrange("p (w c) -> p w c", c=17)
            sums = recp.tile([P, 16], f32, tag="sums",
                             name=f"sums_{gi}{suffix}")
            nc.vector.tensor_copy(sums[:, 0:nw], ogv[:, :, 0])
            rec = recp.tile([P, 16], f32, tag="rec", name=f"rec_{gi}{suffix}")
            nc.vector.reciprocal_approx_fast(rec[:, 0:nw], sums[:, 0:nw])
            attn = state.get(("attn", nh))
            if attn is None:
                attn = atp.tile([P, NC4, 2, P], bf16, tag="attn",
                                name=f"attn_{nh}")
                state[("attn", nh)] = attn
            in0 = og[:, 0:17 * nw].rearrange(
                "p (nc a i c) -> p nc a i c", nc=ncn, a=2, i=2)[:, :, :, :, 1:17]
            in1 = rec[:, 0:nw].rearrange(
                "p (nc a i) -> p nc a i", nc=ncn, a=2)[:, :, :, :, None]
            out = attn[:, nc0:nc0 + ncn, :, 32 * g2:32 * g2 + 32].rearrange(
                "p nc a (i c) -> p nc a i c", i=2)
            nc.vector.tensor_mul(out, in0,
                                 in1.broadcast_to([P, ncn, 2, 2, 16]))

        def setup_tr(nh):
            # window idx 0-3 -> bank-0 cols, idx 4-7 -> bank-1 cols, so one
            # strided copy drains each 4-window bank
            if nh == NH - 1:
                tr = s4p.tile([P, N], f32, tag="s4", name="tr_final")
                t16 = tr[:].bitcast(bf16).rearrange("p (i c) -> p i c", c=P)
            else:
                tr = outp.tile([P, 4, P], f32, tag="outp", name=f"tr_{nh}")
                t16 = tr[:].bitcast(bf16).rearrange(
                    "p a (j c) -> p (a j) c", j=2)
            win = lambda idx: t16[:, (idx // 4) * (8 if nh == NH - 1 else 4)
                                  + idx % 4, :]
            state[("trwin", nh)] = win
            state[("trt16", nh)] = t16
            ctf = ctp.tile([P, 8, P], bf16, tag="ctf", name=f"ctf_{nh}")
            state[("ctf", nh)] = ctf

        def emit_transposes(nh):
            setup_tr(nh)
            emit_tr_sub(nh, range(NC4))

        def emit_tr_sub(nh, nc4s):
            attn = state[("attn", nh)]
            win = state[("trwin", nh)]
            for nc4 in nc4s:
                for a in range(2):
                    nc.tensor.transpose(win(nc4 * 2 + a),
                                        attn[:, nc4, a, :], ident)
            emit_ct_copies(nh, nc4s)

        def emit_ct_copies(nh, nc4s):
            # one batched copy per 4-window group (= per bank)
            t16 = state[("trt16", nh)]
            ctf = state[("ctf", nh)]
            cts = state.setdefault(("cts", nh), {})
            idxs = sorted(nc4 * 2 + a for nc4 in nc4s for a in range(2))
            base = 8 if nh == NH - 1 else 4
            for grp0 in (0, 4):
                grp = [i for i in idxs if grp0 <= i < grp0 + 4]
                if not grp:
                    continue
                lo, hi = min(grp), max(grp)
                src_ap = t16[:, (grp0 // 4) * base + lo % 4:
                             (grp0 // 4) * base + hi % 4 + 1, :]
                nc.vector.tensor_copy(ctf[:, lo:hi + 1, :], src_ap)
            for nc4 in nc4s:
                cts[nc4] = [ctf[:, nc4 * 2, :], ctf[:, nc4 * 2 + 1, :]]

        def emit_proj(nh, nc4s=range(NC4), part=0):
            cts = state[("cts", nh)]
            if part == 0 or ("op", nh) not in state:
                state[("op", nh)] = outp.tile([P, NC4, P], f32, tag="outp",
                                              name=f"op_{nh}")
                state[("ob", nh)] = osb.tile([P, NC4, P], f32, tag="osb",
                                             name=f"ob_{nh}")
            op = state[("op", nh)]
            ob = state[("ob", nh)]
            for nc4 in nc4s:
                sl = op[:, nc4, :]
                nc.tensor.matmul(sl, ones1[:], bor, start=True, stop=False)
                nc.tensor.matmul(sl, cts[nc4][0][:], wog,
                                 start=False, stop=False)
                nc.tensor.matmul(sl, cts[nc4][1][:], wog,
                                 start=False, stop=True)
            lo, hi = min(nc4s), max(nc4s) + 1
            if nh == NH - 1 and part == 0:
                nc.scalar.copy(ob[:, lo:hi, :], op[:, lo:hi, :])
            else:
                nc.vector.tensor_copy(ob[:, lo:hi, :], op[:, lo:hi, :])
            dst = out_d.ap().rearrange("(x c p) d -> x p c d", x=NH, c=NC4)[nh]
            nc.sync.dma_start(dst[:, lo:hi, :], ob[:, lo:hi, :])

        # accumulator emission order: eg-based accs first (their inputs are
        # ready one em-latency earlier at the group boundary)
        EG_ACCS = [a for a in range(16) if (a % 4) // 2 == 0]
        EM_ACCS = [a for a in range(16) if (a % 4) // 2 == 1]

        def emit_s4(gi, mc):
            nh, g2 = groups[gi]
            s4 = s4p.tile([P, N], f32, tag="s4", name=f"s4_{gi}_{mc}")
            t = "a" if g2 < 2 else "b"
            qT = qk_tiles["q" + t]
            kT = qk_tiles["k" + t]
            q0 = (2 * g2) % 4
            for i in range(2):
                qq = 32 * (q0 + i)
                nc.tensor.matmul(
                    s4[:, i * NHF:(i + 1) * NHF],
                    kT[qq:qq + 16, mc * P:(mc + 1) * P],
                    qT[qq:qq + 16, nh * NHF:(nh + 1) * NHF],
                    start=True, stop=True, tile_position=(qq, 0))
            return s4

        def emit_sf(gi, i):
            """Score matmul for half i of the fexp chunk (mc=0), own bank."""
            nh, g2 = groups[gi]
            sf = fxp.tile([P, NHF], f32, tag="sf", name=f"sf_{gi}_{i}")
            q0 = (2 * g2) % 4
            qq = 32 * (q0 + i)
            t = "a" if g2 < 2 else "b"
            nc.tensor.matmul(
                sf[:], qk_tiles["k" + t][qq:qq + 16, 0:P],
                qk_tiles["q" + t][qq:qq + 16, nh * NHF:(nh + 1) * NHF],
                start=True, stop=True, tile_position=(qq, 0))
            return sf

        def emit_exp(gi, mc, s4):
            eg = egp.tile([P, N], bf16, tag=f"eg{mc}", name=f"eg_{gi}_{mc}")
            nc.scalar.activation(eg[:], s4[:], Exp)
            return eg

        def emit_em(gi, mc, eg):
            nh, g2 = groups[gi]
            em = emp.tile([P, N], bf16, tag=f"em{mc}", name=f"em_{gi}_{mc}")
            msk = mask_sb[:, mc, nh, :]
            nc.vector.tensor_mul(
                em[:].rearrange("p (i f) -> p i f", i=2),
                eg[:].rearrange("p (i f) -> p i f", i=2),
                msk[:, None, :].broadcast_to([P, 2, NHF]))
            return em

        for gi in range(NG):
            nh, g2 = groups[gi]
            if gi > 0:
                state[gi - 1]["og"] = ogp.tile([P, 512], f32, tag="og",
                                               name=f"og_{gi - 1}")
            egs = [None] * MC
            ems = [None] * MC
            # fexp chunk (mc=0): half 0 scores + DVE bitcast exp
            eg0 = egp.tile([P, N], bf16, tag="eg0", name=f"eg_{gi}_0")
            sf0 = emit_sf(gi, 0)
            nc.vector.tensor_scalar(eg0[:, 0:NHF].bitcast(u16), sf0[:],
                                    FE_A16, FE_B16,
                                    op0=Alu.mult, op1=Alu.add)
            # chunks 1, 2 on ACT (banks already free at the group boundary)
            s4_1 = emit_s4(gi, 1)
            egs[1] = emit_exp(gi, 1, s4_1)
            s4_2 = emit_s4(gi, 2)
            egs[2] = emit_exp(gi, 2, s4_2)
            # fexp half 1 + em(0)
            sf1 = emit_sf(gi, 1)
            nc.vector.tensor_scalar(eg0[:, NHF:N].bitcast(u16), sf1[:],
                                    FE_A16, FE_B16,
                                    op0=Alu.mult, op1=Alu.add)
            egs[0] = eg0
            ems[0] = emit_em(gi, 0, eg0)
            # first half of previous group's attn@V accumulators
            if gi > 0:
                emit_og_accs(gi - 1, EG_ACCS)
            ems[1] = emit_em(gi, 1, egs[1])
            ems[2] = emit_em(gi, 2, egs[2])
            if gi == 0:
                emit_b_projections()
                emit_vp()
            s4_3 = emit_s4(gi, 3)
            egs[3] = emit_exp(gi, 3, s4_3)
            ems[3] = emit_em(gi, 3, egs[3])
            if gi > 0:
                emit_og_accs(gi - 1, EM_ACCS)
                emit_norm(gi - 1)
            if gi == 5:
                emit_transposes(0)
            if gi == 7:
                emit_proj(0)
            for mc in range(4, MC):
                s4 = emit_s4(gi, mc)
                egs[mc] = emit_exp(gi, mc, s4)
                ems[mc] = emit_em(gi, mc, egs[mc])
                if gi == 0:
                    for half in (0, 1):
                        vmc, vsl = vaug_copies[(mc - 4) * 2 + half]
                        nc.vector.tensor_copy(
                            vaug[:, vmc, :, 1:17],
                            vsl.rearrange("p (h c) -> p h c", h=H))
            state[gi] = {"egs": egs, "ems": ems}

        # drain: pipelined across the two og banks (nc4 halves)
        gi = NG - 1
        og7a = ogp.tile([P, 512], f32, tag="og", name="og7a")
        og7b = ogp.tile([P, 512], f32, tag="og", name="og7b")
        emit_og_accs(gi, range(0, 8), og=og7a, wbase=0)      # nc4 0-1
        emit_og_accs(gi, range(8, 16), og=og7b, wbase=8)     # nc4 2-3
        emit_norm(gi, og=og7a, nc0=0, ncn=2, suffix="a")
        setup_tr(NH - 1)
        emit_tr_sub(NH - 1, (0, 1))
        emit_norm(gi, og=og7b, nc0=2, ncn=2, suffix="b")
        emit_proj(NH - 1, nc4s=(0, 1), part=0)
        emit_tr_sub(NH - 1, (2, 3))
        emit_proj(NH - 1, nc4s=(2, 3), part=1)

    nc.compile()
    return nc


def _host_prep(query, key, value, adj_mask, Wq, bq, Wk, bk, Wv, bv, Wo, bo):
    """Per-core input maps (host-side layout transforms only)."""
    f32 = np.float32
    query = np.asarray(query, f32)
    key = np.asarray(key, f32)
    value = np.asarray(value, f32)
    Wq = np.asarray(Wq, f32); Wk = np.asarray(Wk, f32)
    Wv = np.asarray(Wv, f32); Wo = np.asarray(Wo, f32)
    bq = np.asarray(bq, f32); bk = np.asarray(bk, f32)
    bv = np.asarray(bv, f32); bo = np.asarray(bo, f32)
    adj = np.asarray(adj_mask)

    scale = 1.0 / np.sqrt(np.float32(DH))

    def pack_w(Wm):
        out = []
        for t in range(2):
            wt = np.zeros((P, P), f32)
            for j in range(4):
                h = 4 * t + j
                wt[:, 32 * j:32 * j + 16] = Wm[:, DH * h:DH * (h + 1)]
            out.append(wt)
        return out

    wqa, wqb = [w * scale for w in pack_w(Wq)]
    wka, wkb = pack_w(Wk)

    def pack_b(bvec, s):
        cols = []
        for t in range(2):
            col = np.zeros((P,), f32)
            for j in range(4):
                h = 4 * t + j
                col[32 * j:32 * j + 16] = bvec[DH * h:DH * (h + 1)] * s
            cols.append(col)
        return cols

    bqa, bqb = pack_b(bq, scale)
    bka, bkb = pack_b(bk, 1.0)

    c32 = np.zeros((P, 4 * P + 4), f32)
    c32[:, 0:P] = wqa; c32[:, P:2 * P] = wqb
    c32[:, 2 * P:3 * P] = wka; c32[:, 3 * P:4 * P] = wkb
    c32[:, 4 * P + 0] = bqa; c32[:, 4 * P + 1] = bqb
    c32[:, 4 * P + 2] = bka; c32[:, 4 * P + 3] = bkb

    cbf = np.zeros((P, 3 * P), f32)
    cbf[:, 0:P] = Wv
    cbf[:, P:2 * P] = 0.5 * Wo
    cbf[:, 2 * P:3 * P] = np.eye(P, dtype=f32)

    br = np.concatenate([bv, bo]).reshape(1, 2 * P)

    maskT = adj.T.astype(f32)  # [m, n]
    maskL = maskT.reshape(MC, P, NH, NHF).transpose(1, 0, 2, 3).reshape(P, -1)

    shared = {
        "c32": c32,
        "cbf": cbf.astype(_BF16),
        "br": br.astype(_BF16),
        "maskL": maskL.astype(_BF16),
    }
    in_maps = []
    for b in range(B):
        m = dict(shared)
        m["xqT"] = np.ascontiguousarray(query[b].T)
        m["xkT"] = np.ascontiguousarray(key[b].T)
        m["xvT"] = np.ascontiguousarray(value[b].T).astype(_BF16)
        in_maps.append(m)
    return in_maps


def kernel(**inputs):
    if "nc" not in _CACHE:
        _CACHE["nc"] = _build_nc()
    nc = _CACHE["nc"]

    from concourse.bass_utils import run_bass_kernel_spmd

    in_maps = _host_prep(**inputs)
    res = run_bass_kernel_spmd(nc, in_maps, core_ids=list(range(NCORES)))
    out = np.stack([res.results[c]["out"] for c in range(NCORES)], axis=0)
    return out.astype(np.float32)


# revision 4
# speedup vs baseline: 1.0027x; 1.0027x over previous
"""Trainium2 Bass kernel v2 for masked dual-softmax attention.

Reference (per batch b, head h; dh=16, H=8, N=1024, D=128):
  q = query @ Wq + bq ; k = key @ Wk + bk ; v = value @ Wv + bv
  S = q_h k_h^T / sqrt(dh)
  attn = 0.5*(softmax(S) + softmax(S masked by adj))
  out = concat_h(attn @ v_h) @ Wo + bo

Sharding: data-parallel over batch, one batch element per core (8 cores).

Device algorithm (S^T layout: m=key idx on partitions, n=query idx free).
Processing runs in 8 groups of (n-half, head-pair); per group 8 m-chunks of
S^T [128,1024] (2 heads x 512 n):
  - head-permuted projection weight tiles put head 4t+j's dh=16 rows on SBUF
    partition quadrant 32j of tile t, so score matmuls pack 2 heads per PSUM
    chunk via tile_position; q/k bias adds are fused into the mandatory
    PSUM->SBUF copies (split ACT/DVE, staged through spare 1-bank PSUM slots
    so the score-chunk bank rotation is never blocked at the start)
  - exp: 7 chunks/group on ACT (the throughput bottleneck), 1 chunk on DVE
    via the bitcast fast-exp exp(x)~=bf16_bits(uint16(x*A+B)) processed as
    two 512-col halves through a dedicated PSUM bank, which keeps the ACT
    exp chain fully decoupled (zero mid-kernel ACT gaps)
  - em = eg*mask on DVE (bf16 2x mode)
  - attn@V transposed: og[n, 17] = sum_mc eg_chunk^T @ [1|v_h]; the 17-wide
    moving operand costs the PE 17 rows instead of 512, and the ones column
    yields the softmax denominators in the same accumulator. 16 sequential
    accumulators per group (the PSUM zero-region rule forbids interleaving
    accumulation groups within one bank); emitted interleaved with the next
    group's score matmuls to keep the PE fed
  - normalize on DVE: reciprocal_approx_fast on the strided denominator
    column + one fused strided multiply into attn[n, k] tiles
  - [n,k] -> [k,n] via PE identity-matmul transposes striped across freed
    PSUM banks + batched per-bank DVE copies
  - output projection: out[n,d] = catT_g^T (0.5 Wo) + catT_m^T (0.5 Wo) + bo
  - drain is pipelined: the last group's accumulators split across both og
    banks so norm/transpose/projection/store of the first n-half overlap
    the second half's accumulation
"""

import sys

if "/opt/trn_rl_repo" not in sys.path:
    sys.path.insert(0, "/opt/trn_rl_repo")

import numpy as np
import ml_dtypes
from contextlib import ExitStack

B, N, D, H, DH = 8, 1024, 128, 8, 16
NCORES = 8
P = 128
NH = 2          # n halves of 512
NHF = N // NH   # 512
MC = 8          # m chunks of 128
G2 = 4          # head groups of 2
NC4 = 4         # n chunks of 128 per half
NG = NH * G2    # 8 groups

_BF16 = ml_dtypes.bfloat16
_CACHE = {}

# exp schedule: "act" (native exp) or "dve" (bitcast fast-exp; DVE only --
# GPSIMD cannot read PSUM). mask schedule: "dve" or "pool".
EXP_SCHEDULE = {(g, mc): "act" for g in range(NG) for mc in range(MC)}
for _g in range(NG):
    EXP_SCHEDULE[(_g, 0)] = "dve"

# fast-exp: exp(x) ~= bf16_bits( uint16( x*A16 + B16 ) )
# (top 16 bits of the classic int32 bitcast trick; uint16 write rounds)
FE_A16 = float(2 ** 23 / np.log(2.0) / 65536.0)
FE_B16 = float((127 * 2 ** 23 - 366000.0) / 65536.0)


def _build_nc():
    import concourse.tile as tile
    import concourse.mybir as mybir
    from concourse import bacc

    bf16 = mybir.dt.bfloat16
    f32 = mybir.dt.float32
    f32r = mybir.dt.float32r
    u16 = mybir.dt.uint16
    Exp = mybir.ActivationFunctionType.Exp
    Alu = mybir.AluOpType

    nc = bacc.Bacc("TRN2", target_bir_lowering=False, debug=False,
                   num_devices=NCORES)

    # ---- DRAM I/O -------------------------------------------------------
    # c32: wqa|wqb|wka|wkb (4x128 cols) + bqa|bqb|bka|bkb (4 cols)
    c32_d = nc.dram_tensor("c32", [P, 4 * P + 4], f32r, kind="ExternalInput")
    # cbf: wv|wog|identity (3x128 cols)
    cbf_d = nc.dram_tensor("cbf", [P, 3 * P], bf16, kind="ExternalInput")
    # br: bvr|bor rows
    br_d = nc.dram_tensor("br", [1, 2 * P], bf16, kind="ExternalInput")
    xq_d = nc.dram_tensor("xqT", [P, N], f32r, kind="ExternalInput")
    xk_d = nc.dram_tensor("xkT", [P, N], f32r, kind="ExternalInput")
    xv_d = nc.dram_tensor("xvT", [P, N], bf16, kind="ExternalInput")
    mask_d = nc.dram_tensor("maskL", [P, MC * NH * NHF], bf16, kind="ExternalInput")
    out_d = nc.dram_tensor("out", [N, D], f32, kind="ExternalOutput")

    with tile.TileContext(nc) as tc, ExitStack() as ctx:
        const = ctx.enter_context(tc.tile_pool(name="const", bufs=1))
        xpool = ctx.enter_context(tc.tile_pool(name="x", bufs=1))
        qkpool = ctx.enter_context(tc.tile_pool(name="qk", bufs=1))
        egp = ctx.enter_context(tc.tile_pool(name="eg", bufs=3))
        emp = ctx.enter_context(tc.tile_pool(name="em", bufs=3))
        recp = ctx.enter_context(tc.tile_pool(name="rec", bufs=2))
        atp = ctx.enter_context(tc.tile_pool(name="attn", bufs=2))
        ctp = ctx.enter_context(tc.tile_pool(name="catT", bufs=2))
        osb = ctx.enter_context(tc.tile_pool(name="osb", bufs=2))
        # PSUM: s4 2 banks x2, og 1 bank x2, outp 1 bank, fexp 1 bank = 8
        s4p = ctx.enter_context(tc.tile_pool(name="s4", bufs=2, space="PSUM"))
        ogp = ctx.enter_context(tc.tile_pool(name="og", bufs=2, space="PSUM"))
        outp = ctx.enter_context(tc.tile_pool(name="outp", bufs=1, space="PSUM"))
        fxp = ctx.enter_context(tc.tile_pool(name="fxp", bufs=1, space="PSUM"))

        # ---- constants / inputs (order = DMA issue order) ----------------
        c32 = const.tile([P, 4 * P + 4], f32r, tag="c32")
        nc.sync.dma_start(c32[:], c32_d.ap())
        xq = xpool.tile([P, N], f32r, tag="xq")
        xk = xpool.tile([P, N], f32r, tag="xk")
        xv = xpool.tile([P, N], bf16, tag="xv")
        nc.sync.dma_start(xq[:], xq_d.ap())
        nc.sync.dma_start(xk[:], xk_d.ap())
        nc.sync.dma_start(xv[:], xv_d.ap())
        cbf = const.tile([P, 3 * P], bf16, tag="cbf")
        nc.sync.dma_start(cbf[:], cbf_d.ap())
        br = const.tile([1, 2 * P], bf16, tag="br")
        nc.sync.dma_start(br[:], br_d.ap())
        mask_sb = const.tile([P, MC, NH, NHF], bf16, tag="mask")
        mask_dr = mask_d.ap().rearrange("p (a b f) -> p a b f", a=MC, b=NH)
        for _mc in range(MC):
            nc.sync.dma_start(mask_sb[:, _mc, :, :], mask_dr[:, _mc, :, :])

        wqa, wqb = c32[:, 0:P], c32[:, P:2 * P]
        wka, wkb = c32[:, 2 * P:3 * P], c32[:, 3 * P:4 * P]
        bias_col = {"qa": c32[:, 4 * P + 0:4 * P + 1].bitcast(f32),
                    "qb": c32[:, 4 * P + 1:4 * P + 2].bitcast(f32),
                    "ka": c32[:, 4 * P + 2:4 * P + 3].bitcast(f32),
                    "kb": c32[:, 4 * P + 3:4 * P + 4].bitcast(f32)}
        wv, wog, ident = cbf[:, 0:P], cbf[:, P:2 * P], cbf[:, 2 * P:3 * P]
        bvr, bor = br[:, 0:P], br[:, P:2 * P]

        ones1 = const.tile([1, P], bf16, tag="ones1")
        nc.vector.memset(ones1[:], 1.0)

        # ---- PE warm-up: ramp the p-state while input DMAs land ----------
        warm = ogp.tile([P, 512], f32, tag="og", name="warm")
        for _w in range(28):
            nc.tensor.matmul(warm[:, 0:P], ones1[:], ones1[:],
                             start=True, stop=True)

        # ---- q/k projections (head-permuted quadrant packing) ------------
        # qa/ka use the two 2-bank s4 slots (freed ASAP for the score
        # chunks); qb/kb go through the 1-bank fxp / og slots in halves so
        # they never block the s4 rotation.
        qk_tiles = {}
        for name in ("qa", "ka", "qb", "kb"):
            qk_tiles[name] = qkpool.tile([P, N], f32r, tag=name, name=name)
        # qa: n-half 1 through the fxp bank, copied on ACT (fills early ACT
        # idle; covers all nh=0 score chunks); n-half 2 via og bank on DVE
        qah1 = fxp.tile([P, NHF], f32, tag="sf", name="qah1")
        nc.tensor.matmul(qah1[:], wqa, xq[:, 0:NHF], start=True, stop=True)
        nc.scalar.add(qk_tiles["qa"][:, 0:NHF], qah1[:], bias_col["qa"])
        qah2 = ogp.tile([P, 512], f32, tag="og", name="qah2")
        nc.tensor.matmul(qah2[:], wqa, xq[:, NHF:N], start=True, stop=True)
        nc.vector.tensor_scalar_add(qk_tiles["qa"][:, NHF:N], qah2[:],
                                    bias_col["qa"])
        # ka: s4-pool psum; m-cols 0:256 copied first (unblocks s4(0,1))
        ps_ka = s4p.tile([P, N], f32, tag="s4", name="proj_ka")
        for s in range(NH):
            nc.tensor.matmul(ps_ka[:, s * NHF:(s + 1) * NHF], wka,
                             xk[:, s * NHF:(s + 1) * NHF],
                             start=True, stop=True)
        nc.vector.tensor_scalar_add(qk_tiles["ka"][:, 0:256],
                                    ps_ka[:, 0:256], bias_col["ka"])
        nc.vector.tensor_scalar_add(qk_tiles["ka"][:, 256:N],
                                    ps_ka[:, 256:N], bias_col["ka"])
        def emit_b_projections():
            # qb: half 1 in the fxp bank, half 2 in an og bank
            qbh1 = fxp.tile([P, NHF], f32, tag="sf", name="qbh1")
            nc.tensor.matmul(qbh1[:], wqb, xq[:, 0:NHF], start=True, stop=True)
            nc.vector.tensor_scalar_add(qk_tiles["qb"][:, 0:NHF], qbh1[:],
                                        bias_col["qb"])
            qbh2 = ogp.tile([P, 512], f32, tag="og", name="qbh2")
            nc.tensor.matmul(qbh2[:], wqb, xq[:, NHF:N], start=True, stop=True)
            nc.vector.tensor_scalar_add(qk_tiles["qb"][:, NHF:N], qbh2[:],
                                        bias_col["qb"])
            # kb: two og-bank halves
            kbh1 = ogp.tile([P, 512], f32, tag="og", name="kbh1")
            nc.tensor.matmul(kbh1[:], wkb, xk[:, 0:NHF], start=True, stop=True)
            nc.vector.tensor_scalar_add(qk_tiles["kb"][:, 0:NHF], kbh1[:],
                                        bias_col["kb"])
            kbh2 = ogp.tile([P, 512], f32, tag="og", name="kbh2")
            nc.tensor.matmul(kbh2[:], wkb, xk[:, NHF:N], start=True, stop=True)
            nc.vector.tensor_scalar_add(qk_tiles["kb"][:, NHF:N], kbh2[:],
                                        bias_col["kb"])

        # ---- v projection: natural layout [m, dim], augmented [1|v] ------
        # vaug[p, mc, h, 0] = 1 ; vaug[p, mc, h, 1+c] = V[mc*128+p, 16h+c]
        vaug = qkpool.tile([P, MC, H, 17], bf16, tag="vaug")
        nc.vector.memset(vaug[:, :, :, 0], 1.0)

        vaug_copies = []

        def emit_vp():
            for half in range(2):
                vp = outp.tile([P, 4, P], f32, tag="outp", name=f"vps{half}")
                for mc4 in range(4):
                    mc = half * 4 + mc4
                    sl = vp[:, mc4, :]
                    nc.tensor.matmul(sl, ones1[:], bvr,
                                     start=True, stop=False)
                    nc.tensor.matmul(sl, xv[:, mc * P:(mc + 1) * P], wv,
                                     start=False, stop=True)
                    vaug_copies.append((mc, sl))

        # ---- main loop --------------------------------------------------
        groups = [(nh, g2) for nh in range(NH) for g2 in range(G2)]
        state = {}

        def emit_og_accs(gi, accs, og=None, wbase=0):
            """Emit the given accumulator indices (0..15) of group gi."""
            nh, g2 = groups[gi]
            st = state[gi]
            og = og if og is not None else st["og"]
            for acc in accs:
                nc4, r = divmod(acc, 4)
                a, i = divmod(r, 2)
                tiles = st["egs"] if a == 0 else st["ems"]
                w = (acc - wbase) * 17
                for mc in range(MC):
                    nc.tensor.matmul(
                        og[:, w:w + 17],
                        tiles[mc][:, i * NHF + nc4 * P:i * NHF + (nc4 + 1) * P],
                        vaug[:, mc, 2 * g2 + i, :],
                        start=(mc == 0), stop=(mc == MC - 1))

        def emit_norm(gi, og=None, nc0=0, ncn=NC4, suffix=""):
            """Normalize accumulators for nc4 range [nc0, nc0+ncn) held in
            `og` (windows start at (nc4-nc0)*68 cols)."""
            nh, g2 = groups[gi]
            st = state[gi]
            og = og if og is not None else st["og"]
            nw = ncn * 4
            ogv = og[:, 0:17 * nw].rearrange("p (w c) -> p w c", c=17)
            rec = recp.tile([P, 16], f32, tag="rec", name=f"rec_{gi}{suffix}")
            nc.vector.reciprocal_approx_fast(rec[:, 0:nw], ogv[:, :, 0])
            attn = state.get(("attn", nh))
            if attn is None:
                attn = atp.tile([P, NC4, 2, P], bf16, tag="attn",
                                name=f"attn_{nh}")
                state[("attn", nh)] = attn
            in0 = og[:, 0:17 * nw].rearrange(
                "p (nc a i c) -> p nc a i c", nc=ncn, a=2, i=2)[:, :, :, :, 1:17]
            in1 = rec[:, 0:nw].rearrange(
                "p (nc a i) -> p nc a i", nc=ncn, a=2)[:, :, :, :, None]
            out = attn[:, nc0:nc0 + ncn, :, 32 * g2:32 * g2 + 32].rearrange(
                "p nc a (i c) -> p nc a i c", i=2)
            nc.vector.tensor_mul(out, in0,
                                 in1.broadcast_to([P, ncn, 2, 2, 16]))

        def setup_tr(nh):
            # window idx 0-3 -> bank-0 cols, idx 4-7 -> bank-1 cols, so one
            # strided copy drains each 4-window bank
            if nh == NH - 1:
                tr = s4p.tile([P, N], f32, tag="s4", name="tr_final")
                t16 = tr[:].bitcast(bf16).rearrange("p (i c) -> p i c", c=P)
            else:
                tr = outp.tile([P, 4, P], f32, tag="outp", name=f"tr_{nh}")
                t16 = tr[:].bitcast(bf16).rearrange(
                    "p a (j c) -> p (a j) c", j=2)
            win = lambda idx: t16[:, (idx // 4) * (8 if nh == NH - 1 else 4)
                                  + idx % 4, :]
            state[("trwin", nh)] = win
            state[("trt16", nh)] = t16
            ctf = ctp.tile([P, 8, P], bf16, tag="ctf", name=f"ctf_{nh}")
            state[("ctf", nh)] = ctf

        def emit_transposes(nh):
            setup_tr(nh)
            emit_tr_sub(nh, range(NC4))

        def emit_tr_sub(nh, nc4s):
            attn = state[("attn", nh)]
            win = state[("trwin", nh)]
            for nc4 in nc4s:
                for a in range(2):
                    nc.tensor.transpose(win(nc4 * 2 + a),
                                        attn[:, nc4, a, :], ident)
            emit_ct_copies(nh, nc4s)

        def emit_ct_copies(nh, nc4s):
            # one batched copy per 4-window group (= per bank)
            t16 = state[("trt16", nh)]
            ctf = state[("ctf", nh)]
            cts = state.setdefault(("cts", nh), {})
            idxs = sorted(nc4 * 2 + a for nc4 in nc4s for a in range(2))
            base = 8 if nh == NH - 1 else 4
            for grp0 in (0, 4):
                grp = [i for i in idxs if grp0 <= i < grp0 + 4]
                if not grp:
                    continue
                lo, hi = min(grp), max(grp)
                src_ap = t16[:, (grp0 // 4) * base + lo % 4:
                             (grp0 // 4) * base + hi % 4 + 1, :]
                nc.vector.tensor_copy(ctf[:, lo:hi + 1, :], src_ap)
            for nc4 in nc4s:
                cts[nc4] = [ctf[:, nc4 * 2, :], ctf[:, nc4 * 2 + 1, :]]

        def emit_proj(nh, nc4s=range(NC4), part=0):
            cts = state[("cts", nh)]
            if part == 0 or ("op", nh) not in state:
                state[("op", nh)] = outp.tile([P, NC4, P], f32, tag="outp",
                                              name=f"op_{nh}")
                state[("ob", nh)] = osb.tile([P, NC4, P], f32, tag="osb",
                                             name=f"ob_{nh}")
            op = state[("op", nh)]
            ob = state[("ob", nh)]
            for nc4 in nc4s:
                sl = op[:, nc4, :]
                nc.tensor.matmul(sl, ones1[:], bor, start=True, stop=False)
                nc.tensor.matmul(sl, cts[nc4][0][:], wog,
                                 start=False, stop=False)
                nc.tensor.matmul(sl, cts[nc4][1][:], wog,
                                 start=False, stop=True)
            lo, hi = min(nc4s), max(nc4s) + 1
            if nh == NH - 1 and part == 0:
                nc.scalar.copy(ob[:, lo:hi, :], op[:, lo:hi, :])
            else:
                nc.vector.tensor_copy(ob[:, lo:hi, :], op[:, lo:hi, :])
            dst = out_d.ap().rearrange("(x c p) d -> x p c d", x=NH, c=NC4)[nh]
            nc.sync.dma_start(dst[:, lo:hi, :], ob[:, lo:hi, :])

        # accumulator emission order: eg-based accs first (their inputs are
        # ready one em-latency earlier at the group boundary)
        EG_ACCS = [a for a in range(16) if (a % 4) // 2 == 0]
        EM_ACCS = [a for a in range(16) if (a % 4) // 2 == 1]

        def emit_s4(gi, mc):
            nh, g2 = groups[gi]
            s4 = s4p.tile([P, N], f32, tag="s4", name=f"s4_{gi}_{mc}")
            t = "a" if g2 < 2 else "b"
            qT = qk_tiles["q" + t]
            kT = qk_tiles["k" + t]
            q0 = (2 * g2) % 4
            for i in range(2):
                qq = 32 * (q0 + i)
                nc.tensor.matmul(
                    s4[:, i * NHF:(i + 1) * NHF],
                    kT[qq:qq + 16, mc * P:(mc + 1) * P],
                    qT[qq:qq + 16, nh * NHF:(nh + 1) * NHF],
                    start=True, stop=True, tile_position=(qq, 0))
            return s4

        def emit_sf(gi, mc, i):
            """Score matmul for head-half i of chunk mc via the fxp bank."""
            nh, g2 = groups[gi]
            sf = fxp.tile([P, NHF], f32, tag="sf", name=f"sf_{gi}_{mc}_{i}")
            q0 = (2 * g2) % 4
            qq = 32 * (q0 + i)
            t = "a" if g2 < 2 else "b"
            nc.tensor.matmul(
                sf[:], qk_tiles["k" + t][qq:qq + 16, mc * P:(mc + 1) * P],
                qk_tiles["q" + t][qq:qq + 16, nh * NHF:(nh + 1) * NHF],
                start=True, stop=True, tile_position=(qq, 0))
            return sf

        def emit_exp(gi, mc, s4):
            eg = egp.tile([P, N], bf16, tag=f"eg{mc}", name=f"eg_{gi}_{mc}")
            nc.scalar.activation(eg[:], s4[:], Exp)
            return eg

        def emit_em(gi, mc, eg):
            nh, g2 = groups[gi]
            em = emp.tile([P, N], bf16, tag=f"em{mc}", name=f"em_{gi}_{mc}")
            msk = mask_sb[:, mc, nh, :]
            nc.vector.tensor_mul(
                em[:].rearrange("p (i f) -> p i f", i=2),
                eg[:].rearrange("p (i f) -> p i f", i=2),
                msk[:, None, :].broadcast_to([P, 2, NHF]))
            return em

        for gi in range(NG):
            nh, g2 = groups[gi]
            if gi > 0:
                state[gi - 1]["og"] = ogp.tile([P, 512], f32, tag="og",
                                               name=f"og_{gi - 1}")
            egs = [None] * MC
            ems = [None] * MC
            # fexp chunk (mc=0): half 0 scores + DVE bitcast exp
            eg0 = egp.tile([P, N], bf16, tag="eg0", name=f"eg_{gi}_0")
            sf0 = emit_sf(gi, 0, 0)
            nc.vector.tensor_scalar(eg0[:, 0:NHF].bitcast(u16), sf0[:],
                                    FE_A16, FE_B16,
                                    op0=Alu.mult, op1=Alu.add)
            s4_1 = emit_s4(gi, 1)
            egs[1] = emit_exp(gi, 1, s4_1)
            s4_2 = emit_s4(gi, 2)
            egs[2] = emit_exp(gi, 2, s4_2)
            sf1 = emit_sf(gi, 0, 1)
            nc.vector.tensor_scalar(eg0[:, NHF:N].bitcast(u16), sf1[:],
                                    FE_A16, FE_B16,
                                    op0=Alu.mult, op1=Alu.add)
            egs[0] = eg0
            ems[0] = emit_em(gi, 0, eg0)
            # first half of previous group's attn@V accumulators
            if gi > 0:
                emit_og_accs(gi - 1, EG_ACCS)
            ems[1] = emit_em(gi, 1, egs[1])
            ems[2] = emit_em(gi, 2, egs[2])

            if gi == 0:
                emit_b_projections()
                emit_vp()
            s4_3 = emit_s4(gi, 3)
            egs[3] = emit_exp(gi, 3, s4_3)
            ems[3] = emit_em(gi, 3, egs[3])
            if gi > 0:
                emit_og_accs(gi - 1, EM_ACCS)
                emit_norm(gi - 1)
            if gi == 5:
                emit_transposes(0)
            if gi == 7:
                emit_proj(0)
            for mc in range(4, MC):
                s4 = emit_s4(gi, mc)
                egs[mc] = emit_exp(gi, mc, s4)
                ems[mc] = emit_em(gi, mc, egs[mc])
                if gi == 0:
                    for half in (0, 1):
                        vmc, vsl = vaug_copies[(mc - 4) * 2 + half]
                        nc.vector.tensor_copy(
                            vaug[:, vmc, :, 1:17],
                            vsl.rearrange("p (h c) -> p h c", h=H))
            state[gi] = {"egs": egs, "ems": ems}

        # drain: pipelined across the two og banks (nc4 halves)
        gi = NG - 1
        og7a = ogp.tile([P, 512], f32, tag="og", name="og7a")
        og7b = ogp.tile([P, 512], f32, tag="og", name="og7b")
        emit_og_accs(gi, range(0, 8), og=og7a, wbase=0)      # nc4 0-1
        emit_og_accs(gi, range(8, 16), og=og7b, wbase=8)     # nc4 2-3
        emit_norm(gi, og=og7a, nc0=0, ncn=2, suffix="a")
        setup_tr(NH - 1)
        emit_tr_sub(NH - 1, (0, 1))
        emit_norm(gi, og=og7b, nc0=2, ncn=2, suffix="b")
        emit_proj(NH - 1, nc4s=(0, 1), part=0)
        emit_tr_sub(NH - 1, (2, 3))
        emit_proj(NH - 1, nc4s=(2, 3), part=1)

    nc.compile()
    return nc


def _host_prep(query, key, value, adj_mask, Wq, bq, Wk, bk, Wv, bv, Wo, bo):
    """Per-core input maps (host-side layout transforms only)."""
    f32 = np.float32
    query = np.asarray(query, f32)
    key = np.asarray(key, f32)
    value = np.asarray(value, f32)
    Wq = np.asarray(Wq, f32); Wk = np.asarray(Wk, f32)
    Wv = np.asarray(Wv, f32); Wo = np.asarray(Wo, f32)
    bq = np.asarray(bq, f32); bk = np.asarray(bk, f32)
    bv = np.asarray(bv, f32); bo = np.asarray(bo, f32)
    adj = np.asarray(adj_mask)

    scale = 1.0 / np.sqrt(np.float32(DH))

    def pack_w(Wm):
        out = []
        for t in range(2):
            wt = np.zeros((P, P), f32)
            for j in range(4):
                h = 4 * t + j
                wt[:, 32 * j:32 * j + 16] = Wm[:, DH * h:DH * (h + 1)]
            out.append(wt)
        return out

    wqa, wqb = [w * scale for w in pack_w(Wq)]
    wka, wkb = pack_w(Wk)

    def pack_b(bvec, s):
        cols = []
        for t in range(2):
            col = np.zeros((P,), f32)
            for j in range(4):
                h = 4 * t + j
                col[32 * j:32 * j + 16] = bvec[DH * h:DH * (h + 1)] * s
            cols.append(col)
        return cols

    bqa, bqb = pack_b(bq, scale)
    bka, bkb = pack_b(bk, 1.0)

    c32 = np.zeros((P, 4 * P + 4), f32)
    c32[:, 0:P] = wqa; c32[:, P:2 * P] = wqb
    c32[:, 2 * P:3 * P] = wka; c32[:, 3 * P:4 * P] = wkb
    c32[:, 4 * P + 0] = bqa; c32[:, 4 * P + 1] = bqb
    c32[:, 4 * P + 2] = bka; c32[:, 4 * P + 3] = bkb

    cbf = np.zeros((P, 3 * P), f32)
    cbf[:, 0:P] = Wv
    cbf[:, P:2 * P] = 0.5 * Wo
    cbf[:, 2 * P:3 * P] = np.eye(P, dtype=f32)

    br = np.concatenate([bv, bo]).reshape(1, 2 * P)

    maskT = adj.T.astype(f32)  # [m, n]
    maskL = maskT.reshape(MC, P, NH, NHF).transpose(1, 0, 2, 3).reshape(P, -1)

    shared = {
        "c32": c32,
        "cbf": cbf.astype(_BF16),
        "br": br.astype(_BF16),
        "maskL": maskL.astype(_BF16),
    }
    in_maps = []
    for b in range(B):
        m = dict(shared)
        m["xqT"] = np.ascontiguousarray(query[b].T)
        m["xkT"] = np.ascontiguousarray(key[b].T)
        m["xvT"] = np.ascontiguousarray(value[b].T).astype(_BF16)
        in_maps.append(m)
    return in_maps


def kernel(**inputs):
    if "nc" not in _CACHE:
        _CACHE["nc"] = _build_nc()
    nc = _CACHE["nc"]

    from concourse.bass_utils import run_bass_kernel_spmd

    in_maps = _host_prep(**inputs)
    res = run_bass_kernel_spmd(nc, in_maps, core_ids=list(range(NCORES)))
    out = np.stack([res.results[c]["out"] for c in range(NCORES)], axis=0)
    return out.astype(np.float32)


# revision 6
# speedup vs baseline: 1.0192x; 1.0165x over previous
"""Trainium2 Bass kernel v2 for masked dual-softmax attention.

Reference (per batch b, head h; dh=16, H=8, N=1024, D=128):
  q = query @ Wq + bq ; k = key @ Wk + bk ; v = value @ Wv + bv
  S = q_h k_h^T / sqrt(dh)
  attn = 0.5*(softmax(S) + softmax(S masked by adj))
  out = concat_h(attn @ v_h) @ Wo + bo

Sharding: data-parallel over batch, one batch element per core (8 cores).

Device algorithm (S^T layout: m=key idx on partitions, n=query idx free).
Processing runs in 8 groups of (n-half, head-pair); per group 8 m-chunks of
S^T [128,1024] (2 heads x 512 n):
  - head-permuted projection weight tiles put head 4t+j's dh=16 rows on SBUF
    partition quadrant 32j of tile t, so score matmuls pack 2 heads per PSUM
    chunk via tile_position; q/k bias adds are fused into the mandatory
    PSUM->SBUF copies (split ACT/DVE, staged through spare 1-bank PSUM slots
    so the score-chunk bank rotation is never blocked at the start)
  - exp: 7 chunks/group on ACT (the throughput bottleneck), 1 chunk on DVE
    via the bitcast fast-exp exp(x)~=bf16_bits(uint16(x*A+B)) processed as
    two 512-col halves through a dedicated PSUM bank, which keeps the ACT
    exp chain fully decoupled (zero mid-kernel ACT gaps)
  - em = eg*mask on DVE (bf16 2x mode)
  - attn@V transposed: og[n, 17] = sum_mc eg_chunk^T @ [1|v_h]; the 17-wide
    moving operand costs the PE 17 rows instead of 512, and the ones column
    yields the softmax denominators in the same accumulator. 16 sequential
    accumulators per group (the PSUM zero-region rule forbids interleaving
    accumulation groups within one bank); emitted interleaved with the next
    group's score matmuls to keep the PE fed
  - normalize on DVE: reciprocal_approx_fast on the strided denominator
    column + one fused strided multiply into attn[n, k] tiles
  - [n,k] -> [k,n] via PE identity-matmul transposes striped across freed
    PSUM banks + batched per-bank DVE copies
  - output projection: out[n,d] = catT_g^T (0.5 Wo) + catT_m^T (0.5 Wo) + bo
  - drain is pipelined: the last group's accumulators split across both og
    banks so norm/transpose/projection/store of the first n-half overlap
    the second half's accumulation
"""

import sys

if "/opt/trn_rl_repo" not in sys.path:
    sys.path.insert(0, "/opt/trn_rl_repo")

import numpy as np
import ml_dtypes
from contextlib import ExitStack

B, N, D, H, DH = 8, 1024, 128, 8, 16
NCORES = 8
P = 128
NH = 2          # n halves of 512
NHF = N // NH   # 512
MC = 8          # m chunks of 128
G2 = 4          # head groups of 2
NC4 = 4         # n chunks of 128 per half
NG = NH * G2    # 8 groups

_BF16 = ml_dtypes.bfloat16
_CACHE = {}

# exp schedule: "act" (native exp) or "dve" (bitcast fast-exp; DVE only --
# GPSIMD cannot read PSUM). mask schedule: "dve" or "pool".
EXP_SCHEDULE = {(g, mc): "act" for g in range(NG) for mc in range(MC)}
for _g in range(NG):
    EXP_SCHEDULE[(_g, 0)] = "dve"

# fast-exp: exp(x) ~= bf16_bits( uint16( x*A16 + B16 ) )
# (top 16 bits of the classic int32 bitcast trick; uint16 write rounds)
FE_A16 = float(2 ** 23 / np.log(2.0) / 65536.0)
FE_B16 = float((127 * 2 ** 23 - 366000.0) / 65536.0)


def _build_nc():
    import concourse.tile as tile
    import concourse.mybir as mybir
    from concourse import bacc

    bf16 = mybir.dt.bfloat16
    f32 = mybir.dt.float32
    f32r = mybir.dt.float32r
    u16 = mybir.dt.uint16
    Exp = mybir.ActivationFunctionType.Exp
    Alu = mybir.AluOpType

    nc = bacc.Bacc("TRN2", target_bir_lowering=False, debug=False,
                   num_devices=NCORES)

    # ---- DRAM I/O -------------------------------------------------------
    # cw: wqa|wqb|wka|wkb (4x128 cols) + bqa|bqb|bka|bkb (4 f32 cols
    # stored as 8 bf16 columns bit-cast)
    cw_d = nc.dram_tensor("cw", [P, 4 * P + 8], bf16, kind="ExternalInput")
    # cbf: wv|wog|identity (3x128 cols)
    cbf_d = nc.dram_tensor("cbf", [P, 3 * P], bf16, kind="ExternalInput")
    # br: bvr|bor rows
    br_d = nc.dram_tensor("br", [1, 2 * P], bf16, kind="ExternalInput")
    xq_d = nc.dram_tensor("xqT", [P, N], bf16, kind="ExternalInput")
    xk_d = nc.dram_tensor("xkT", [P, N], bf16, kind="ExternalInput")
    xv_d = nc.dram_tensor("xvT", [P, N], bf16, kind="ExternalInput")
    mask_d = nc.dram_tensor("maskL", [P, MC * NH * NHF], bf16, kind="ExternalInput")
    out_d = nc.dram_tensor("out", [N, D], f32, kind="ExternalOutput")

    with tile.TileContext(nc) as tc, ExitStack() as ctx:
        const = ctx.enter_context(tc.tile_pool(name="const", bufs=1))
        xpool = ctx.enter_context(tc.tile_pool(name="x", bufs=1))
        qkpool = ctx.enter_context(tc.tile_pool(name="qk", bufs=1))
        egp = ctx.enter_context(tc.tile_pool(name="eg", bufs=3))
        emp = ctx.enter_context(tc.tile_pool(name="em", bufs=3))
        recp = ctx.enter_context(tc.tile_pool(name="rec", bufs=2))
        atp = ctx.enter_context(tc.tile_pool(name="attn", bufs=2))
        ctp = ctx.enter_context(tc.tile_pool(name="catT", bufs=2))
        osb = ctx.enter_context(tc.tile_pool(name="osb", bufs=2))
        # PSUM: s4 2 banks x2, og 1 bank x2, outp 1 bank, fexp 1 bank = 8
        s4p = ctx.enter_context(tc.tile_pool(name="s4", bufs=2, space="PSUM"))
        ogp = ctx.enter_context(tc.tile_pool(name="og", bufs=2, space="PSUM"))
        outp = ctx.enter_context(tc.tile_pool(name="outp", bufs=1, space="PSUM"))
        fxp = ctx.enter_context(tc.tile_pool(name="fxp", bufs=1, space="PSUM"))

        # ---- constants / inputs (order = DMA issue order) ----------------
        cw = const.tile([P, 4 * P + 8], bf16, tag="cw")
        nc.sync.dma_start(cw[:], cw_d.ap())
        xq = xpool.tile([P, N], bf16, tag="xq")
        xk = xpool.tile([P, N], bf16, tag="xk")
        xv = xpool.tile([P, N], bf16, tag="xv")
        nc.sync.dma_start(xq[:], xq_d.ap())
        nc.sync.dma_start(xk[:], xk_d.ap())
        nc.sync.dma_start(xv[:], xv_d.ap())
        cbf = const.tile([P, 3 * P], bf16, tag="cbf")
        nc.sync.dma_start(cbf[:], cbf_d.ap())
        br = const.tile([1, 2 * P], bf16, tag="br")
        nc.sync.dma_start(br[:], br_d.ap())
        mask_sb = const.tile([P, MC, NH, NHF], bf16, tag="mask")
        mask_dr = mask_d.ap().rearrange("p (a b f) -> p a b f", a=MC, b=NH)
        for _mc in range(MC):
            nc.sync.dma_start(mask_sb[:, _mc, :, :], mask_dr[:, _mc, :, :])

        wqa, wqb = cw[:, 0:P], cw[:, P:2 * P]
        wka, wkb = cw[:, 2 * P:3 * P], cw[:, 3 * P:4 * P]
        bias32 = cw[:, 4 * P:4 * P + 8].bitcast(f32)
        bias_col = {"qa": bias32[:, 0:1], "qb": bias32[:, 1:2],
                    "ka": bias32[:, 2:3], "kb": bias32[:, 3:4]}
        wv, wog, ident = cbf[:, 0:P], cbf[:, P:2 * P], cbf[:, 2 * P:3 * P]
        bvr, bor = br[:, 0:P], br[:, P:2 * P]

        ones1 = const.tile([1, P], bf16, tag="ones1")
        nc.vector.memset(ones1[:], 1.0)
        # tiny dummy activation: anchors the one-time ACT table load at t~0
        # (otherwise it attaches to the first real activation mid-head)
        dummy = const.tile([1, 1], bf16, tag="dummy")
        nc.scalar.activation(dummy[:], ones1[0:1, 0:1],
                             mybir.ActivationFunctionType.Exp)

        # ---- PE warm-up: ramp the p-state while input DMAs land ----------
        warm = ogp.tile([P, 512], f32, tag="og", name="warm")
        for _w in range(24):
            nc.tensor.matmul(warm[:, 0:P], ones1[:], ones1[:],
                             start=True, stop=True)

        # ---- q/k projections (head-permuted quadrant packing) ------------
        # qa/ka use the two 2-bank s4 slots (freed ASAP for the score
        # chunks); qb/kb go through the 1-bank fxp / og slots in halves so
        # they never block the s4 rotation.
        qk_tiles = {}
        for name in ("qa", "ka", "qb", "kb"):
            qk_tiles[name] = qkpool.tile([P, N], f32r, tag=name, name=name)
        # qa: n-half 1 through the fxp bank, copied on ACT (fills early ACT
        # idle; covers all nh=0 score chunks); n-half 2 via og bank on DVE
        qah1 = fxp.tile([P, NHF], f32, tag="sf", name="qah1")
        nc.tensor.matmul(qah1[:], wqa, xq[:, 0:NHF], start=True, stop=True)
        nc.scalar.add(qk_tiles["qa"][:, 0:NHF], qah1[:], bias_col["qa"])
        qah2 = ogp.tile([P, 512], f32, tag="og", name="qah2")
        nc.tensor.matmul(qah2[:], wqa, xq[:, NHF:N], start=True, stop=True)
        nc.scalar.add(qk_tiles["qa"][:, NHF:N], qah2[:], bias_col["qa"])
        # ka: s4-pool psum; m-cols 0:256 copied first (unblocks s4(0,1))
        ps_ka = s4p.tile([P, N], f32, tag="s4", name="proj_ka")
        for s in range(NH):
            nc.tensor.matmul(ps_ka[:, s * NHF:(s + 1) * NHF], wka,
                             xk[:, s * NHF:(s + 1) * NHF],
                             start=True, stop=True)
        nc.vector.tensor_scalar_add(qk_tiles["ka"][:, 0:256],
                                    ps_ka[:, 0:256], bias_col["ka"])
        nc.vector.tensor_scalar_add(qk_tiles["ka"][:, 256:N],
                                    ps_ka[:, 256:N], bias_col["ka"])
        def emit_b_projections():
            # qb: half 1 in the fxp bank, half 2 in an og bank
            qbh1 = fxp.tile([P, NHF], f32, tag="sf", name="qbh1")
            nc.tensor.matmul(qbh1[:], wqb, xq[:, 0:NHF], start=True, stop=True)
            nc.vector.tensor_scalar_add(qk_tiles["qb"][:, 0:NHF], qbh1[:],
                                        bias_col["qb"])
            qbh2 = ogp.tile([P, 512], f32, tag="og", name="qbh2")
            nc.tensor.matmul(qbh2[:], wqb, xq[:, NHF:N], start=True, stop=True)
            nc.vector.tensor_scalar_add(qk_tiles["qb"][:, NHF:N], qbh2[:],
                                        bias_col["qb"])
            # kb: two og-bank halves
            kbh1 = ogp.tile([P, 512], f32, tag="og", name="kbh1")
            nc.tensor.matmul(kbh1[:], wkb, xk[:, 0:NHF], start=True, stop=True)
            nc.vector.tensor_scalar_add(qk_tiles["kb"][:, 0:NHF], kbh1[:],
                                        bias_col["kb"])
            kbh2 = ogp.tile([P, 512], f32, tag="og", name="kbh2")
            nc.tensor.matmul(kbh2[:], wkb, xk[:, NHF:N], start=True, stop=True)
            nc.vector.tensor_scalar_add(qk_tiles["kb"][:, NHF:N], kbh2[:],
                                        bias_col["kb"])

        # ---- v projection: natural layout [m, dim], augmented [1|v] ------
        # vaug[p, mc, h, 0] = 1 ; vaug[p, mc, h, 1+c] = V[mc*128+p, 16h+c]
        vaug = qkpool.tile([P, MC, H, 17], bf16, tag="vaug")
        nc.vector.memset(vaug[:, :, :, 0], 1.0)

        vaug_copies = []

        def emit_vp():
            for half in range(2):
                vp = outp.tile([P, 4, P], f32, tag="outp", name=f"vps{half}")
                for mc4 in range(4):
                    mc = half * 4 + mc4
                    sl = vp[:, mc4, :]
                    nc.tensor.matmul(sl, ones1[:], bvr,
                                     start=True, stop=False)
                    nc.tensor.matmul(sl, xv[:, mc * P:(mc + 1) * P], wv,
                                     start=False, stop=True)
                vaug_copies.append(vp[:])

        # ---- main loop --------------------------------------------------
        groups = [(nh, g2) for nh in range(NH) for g2 in range(G2)]
        state = {}

        def emit_og_accs(gi, accs, og=None, wbase=0):
            """Emit the given accumulator indices (0..15) of group gi."""
            nh, g2 = groups[gi]
            st = state[gi]
            og = og if og is not None else st["og"]
            for acc in accs:
                nc4, r = divmod(acc, 4)
                a, i = divmod(r, 2)
                tiles = st["egs"] if a == 0 else st["ems"]
                w = (acc - wbase) * 17
                for mc in range(MC):
                    nc.tensor.matmul(
                        og[:, w:w + 17],
                        tiles[mc][:, i * NHF + nc4 * P:i * NHF + (nc4 + 1) * P],
                        vaug[:, mc, 2 * g2 + i, :],
                        start=(mc == 0), stop=(mc == MC - 1))

        def emit_norm(gi, og=None, nc0=0, ncn=NC4, suffix=""):
            """Normalize accumulators for nc4 range [nc0, nc0+ncn) held in
            `og` (windows start at (nc4-nc0)*68 cols)."""
            nh, g2 = groups[gi]
            st = state[gi]
            og = og if og is not None else st["og"]
            nw = ncn * 4
            ogv = og[:, 0:17 * nw].rearrange("p (w c) -> p w c", c=17)
            rec = recp.tile([P, 16], f32, tag="rec", name=f"rec_{gi}{suffix}")
            nc.vector.reciprocal_approx_fast(rec[:, 0:nw], ogv[:, :, 0])
            attn = state.get(("attn", nh))
            if attn is None:
                attn = atp.tile([P, NC4, 2, P], bf16, tag="attn",
                                name=f"attn_{nh}")
                state[("attn", nh)] = attn
            in0 = og[:, 0:17 * nw].rearrange(
                "p (nc a i c) -> p nc a i c", nc=ncn, a=2, i=2)[:, :, :, :, 1:17]
            in1 = rec[:, 0:nw].rearrange(
                "p (nc a i) -> p nc a i", nc=ncn, a=2)[:, :, :, :, None]
            out = attn[:, nc0:nc0 + ncn, :, 32 * g2:32 * g2 + 32].rearrange(
                "p nc a (i c) -> p nc a i c", i=2)
            nc.vector.tensor_mul(out, in0,
                                 in1.broadcast_to([P, ncn, 2, 2, 16]))

        def setup_tr(nh):
            # window idx 0-3 -> bank-0 cols, idx 4-7 -> bank-1 cols, so one
            # strided copy drains each 4-window bank
            if nh == NH - 1:
                tr = s4p.tile([P, N], f32, tag="s4", name="tr_final")
                t16 = tr[:].bitcast(bf16).rearrange("p (i c) -> p i c", c=P)
            else:
                tr = outp.tile([P, 4, P], f32, tag="outp", name=f"tr_{nh}")
                t16 = tr[:].bitcast(bf16).rearrange(
                    "p a (j c) -> p (a j) c", j=2)
            win = lambda idx: t16[:, (idx // 4) * (8 if nh == NH - 1 else 4)
                                  + idx % 4, :]
            state[("trwin", nh)] = win
            state[("trt16", nh)] = t16
            ctf = ctp.tile([P, 8, P], bf16, tag="ctf", name=f"ctf_{nh}")
            state[("ctf", nh)] = ctf

        def emit_transposes(nh):
            setup_tr(nh)
            emit_tr_sub(nh, range(NC4))

        def emit_tr_sub(nh, nc4s):
            attn = state[("attn", nh)]
            win = state[("trwin", nh)]
            for nc4 in nc4s:
                for a in range(2):
                    nc.tensor.transpose(win(nc4 * 2 + a),
                                        attn[:, nc4, a, :], ident)
            emit_ct_copies(nh, nc4s)

        def emit_ct_copies(nh, nc4s):
            # one batched copy per 4-window group (= per bank)
            t16 = state[("trt16", nh)]
            ctf = state[("ctf", nh)]
            cts = state.setdefault(("cts", nh), {})
            idxs = sorted(nc4 * 2 + a for nc4 in nc4s for a in range(2))
            base = 8 if nh == NH - 1 else 4
            for grp0 in (0, 4):
                grp = [i for i in idxs if grp0 <= i < grp0 + 4]
                if not grp:
                    continue
                lo, hi = min(grp), max(grp)
                src_ap = t16[:, (grp0 // 4) * base + lo % 4:
                             (grp0 // 4) * base + hi % 4 + 1, :]
                nc.vector.tensor_copy(ctf[:, lo:hi + 1, :], src_ap)
            for nc4 in nc4s:
                cts[nc4] = [ctf[:, nc4 * 2, :], ctf[:, nc4 * 2 + 1, :]]

        def emit_proj(nh, nc4s=range(NC4), part=0):
            cts = state[("cts", nh)]
            if part == 0 or ("op", nh) not in state:
                state[("op", nh)] = outp.tile([P, NC4, P], f32, tag="outp",
                                              name=f"op_{nh}")
                state[("ob", nh)] = osb.tile([P, NC4, P], f32, tag="osb",
                                             name=f"ob_{nh}")
            op = state[("op", nh)]
            ob = state[("ob", nh)]
            for nc4 in nc4s:
                sl = op[:, nc4, :]
                nc.tensor.matmul(sl, ones1[:], bor, start=True, stop=False)
                nc.tensor.matmul(sl, cts[nc4][0][:], wog,
                                 start=False, stop=False)
                nc.tensor.matmul(sl, cts[nc4][1][:], wog,
                                 start=False, stop=True)
            lo, hi = min(nc4s), max(nc4s) + 1
            if nh == NH - 1 and part == 0:
                nc.scalar.copy(ob[:, lo:hi, :], op[:, lo:hi, :])
            else:
                nc.vector.tensor_copy(ob[:, lo:hi, :], op[:, lo:hi, :])
            dst = out_d.ap().rearrange("(x c p) d -> x p c d", x=NH, c=NC4)[nh]
            nc.sync.dma_start(dst[:, lo:hi, :], ob[:, lo:hi, :])

        # accumulator emission order: eg-based accs first (their inputs are
        # ready one em-latency earlier at the group boundary)
        EG_ACCS = [a for a in range(16) if (a % 4) // 2 == 0]
        EM_ACCS = [a for a in range(16) if (a % 4) // 2 == 1]

        def emit_s4(gi, mc):
            nh, g2 = groups[gi]
            s4 = s4p.tile([P, N], f32, tag="s4", name=f"s4_{gi}_{mc}")
            t = "a" if g2 < 2 else "b"
            qT = qk_tiles["q" + t]
            kT = qk_tiles["k" + t]
            q0 = (2 * g2) % 4
            for i in range(2):
                qq = 32 * (q0 + i)
                nc.tensor.matmul(
                    s4[:, i * NHF:(i + 1) * NHF],
                    kT[qq:qq + 16, mc * P:(mc + 1) * P],
                    qT[qq:qq + 16, nh * NHF:(nh + 1) * NHF],
                    start=True, stop=True, tile_position=(qq, 0))
            return s4

        def emit_sf(gi, mc, i):
            """Score matmul for head-half i of chunk mc via the fxp bank."""
            nh, g2 = groups[gi]
            sf = fxp.tile([P, NHF], f32, tag="sf", name=f"sf_{gi}_{mc}_{i}")
            q0 = (2 * g2) % 4
            qq = 32 * (q0 + i)
            t = "a" if g2 < 2 else "b"
            nc.tensor.matmul(
                sf[:], qk_tiles["k" + t][qq:qq + 16, mc * P:(mc + 1) * P],
                qk_tiles["q" + t][qq:qq + 16, nh * NHF:(nh + 1) * NHF],
                start=True, stop=True, tile_position=(qq, 0))
            return sf

        def emit_exp(gi, mc, s4):
            eg = egp.tile([P, N], bf16, tag=f"eg{mc}", name=f"eg_{gi}_{mc}")
            nc.scalar.activation(eg[:], s4[:], Exp)
            return eg

        def emit_em(gi, mc, eg):
            nh, g2 = groups[gi]
            em = emp.tile([P, N], bf16, tag=f"em{mc}", name=f"em_{gi}_{mc}")
            msk = mask_sb[:, mc, nh, :]
            nc.vector.tensor_mul(
                em[:].rearrange("p (i f) -> p i f", i=2),
                eg[:].rearrange("p (i f) -> p i f", i=2),
                msk[:, None, :].broadcast_to([P, 2, NHF]))
            return em

        for gi in range(NG):
            nh, g2 = groups[gi]
            if gi > 0:
                state[gi - 1]["og"] = ogp.tile([P, 512], f32, tag="og",
                                               name=f"og_{gi - 1}")
            egs = [None] * MC
            ems = [None] * MC
            # fexp chunk (mc=0): half 0 scores + DVE bitcast exp
            eg0 = egp.tile([P, N], bf16, tag="eg0", name=f"eg_{gi}_0")
            sf0 = emit_sf(gi, 0, 0)
            nc.vector.tensor_scalar(eg0[:, 0:NHF].bitcast(u16), sf0[:],
                                    FE_A16, FE_B16,
                                    op0=Alu.mult, op1=Alu.add)
            s4_1 = emit_s4(gi, 1)
            egs[1] = emit_exp(gi, 1, s4_1)
            s4_2 = emit_s4(gi, 2)
            egs[2] = emit_exp(gi, 2, s4_2)
            sf1 = emit_sf(gi, 0, 1)
            nc.vector.tensor_scalar(eg0[:, NHF:N].bitcast(u16), sf1[:],
                                    FE_A16, FE_B16,
                                    op0=Alu.mult, op1=Alu.add)
            egs[0] = eg0
            ems[0] = emit_em(gi, 0, eg0)
            # first half of previous group's attn@V accumulators
            if gi > 0:
                emit_og_accs(gi - 1, EG_ACCS)
            ems[1] = emit_em(gi, 1, egs[1])
            ems[2] = emit_em(gi, 2, egs[2])

            if gi == 0:
                emit_b_projections()
                emit_vp()
            s4_3 = emit_s4(gi, 3)
            egs[3] = emit_exp(gi, 3, s4_3)
            ems[3] = emit_em(gi, 3, egs[3])
            if gi > 0:
                emit_og_accs(gi - 1, EM_ACCS)
                emit_norm(gi - 1)
            if gi == 5:
                emit_transposes(0)
            if gi == 7:
                emit_proj(0)
            for mc in range(4, MC):
                s4 = emit_s4(gi, mc)
                egs[mc] = emit_exp(gi, mc, s4)
                ems[mc] = emit_em(gi, mc, egs[mc])
                if gi == 0 and mc in (4, 6):
                    half = (mc - 4) // 2
                    vp4 = vaug_copies[half]
                    nc.vector.tensor_copy(
                        vaug[:, half * 4:half * 4 + 4, :, 1:17],
                        vp4.rearrange("p c (h d) -> p c h d", h=H))
            state[gi] = {"egs": egs, "ems": ems}

        # drain: pipelined across the two og banks (nc4 halves)
        gi = NG - 1
        og7a = ogp.tile([P, 512], f32, tag="og", name="og7a")
        og7b = ogp.tile([P, 512], f32, tag="og", name="og7b")
        emit_og_accs(gi, range(0, 8), og=og7a, wbase=0)      # nc4 0-1
        emit_og_accs(gi, range(8, 16), og=og7b, wbase=8)     # nc4 2-3
        emit_norm(gi, og=og7a, nc0=0, ncn=2, suffix="a")
        setup_tr(NH - 1)
        emit_tr_sub(NH - 1, (0, 1))
        emit_norm(gi, og=og7b, nc0=2, ncn=2, suffix="b")
        emit_proj(NH - 1, nc4s=(0, 1), part=0)
        emit_tr_sub(NH - 1, (2, 3))
        emit_proj(NH - 1, nc4s=(2, 3), part=1)

    nc.compile()
    return nc


def _host_prep(query, key, value, adj_mask, Wq, bq, Wk, bk, Wv, bv, Wo, bo):
    """Per-core input maps (host-side layout transforms only)."""
    f32 = np.float32
    query = np.asarray(query, f32)
    key = np.asarray(key, f32)
    value = np.asarray(value, f32)
    Wq = np.asarray(Wq, f32); Wk = np.asarray(Wk, f32)
    Wv = np.asarray(Wv, f32); Wo = np.asarray(Wo, f32)
    bq = np.asarray(bq, f32); bk = np.asarray(bk, f32)
    bv = np.asarray(bv, f32); bo = np.asarray(bo, f32)
    adj = np.asarray(adj_mask)

    scale = 1.0 / np.sqrt(np.float32(DH))

    def pack_w(Wm):
        out = []
        for t in range(2):
            wt = np.zeros((P, P), f32)
            for j in range(4):
                h = 4 * t + j
                wt[:, 32 * j:32 * j + 16] = Wm[:, DH * h:DH * (h + 1)]
            out.append(wt)
        return out

    wqa, wqb = [w * scale for w in pack_w(Wq)]
    wka, wkb = pack_w(Wk)

    def pack_b(bvec, s):
        cols = []
        for t in range(2):
            col = np.zeros((P,), f32)
            for j in range(4):
                h = 4 * t + j
                col[32 * j:32 * j + 16] = bvec[DH * h:DH * (h + 1)] * s
            cols.append(col)
        return cols

    bqa, bqb = pack_b(bq, scale)
    bka, bkb = pack_b(bk, 1.0)

    cw = np.zeros((P, 4 * P), f32)
    cw[:, 0:P] = wqa; cw[:, P:2 * P] = wqb
    cw[:, 2 * P:3 * P] = wka; cw[:, 3 * P:4 * P] = wkb
    bias4 = np.stack([bqa, bqb, bka, bkb], axis=1).astype(f32)  # [P, 4]
    cw_b = np.concatenate([cw.astype(_BF16),
                           bias4.view(_BF16).reshape(P, 8)], axis=1)

    cbf = np.zeros((P, 3 * P), f32)
    cbf[:, 0:P] = Wv
    cbf[:, P:2 * P] = 0.5 * Wo
    cbf[:, 2 * P:3 * P] = np.eye(P, dtype=f32)
    cbf_b = cbf.astype(_BF16)

    br = np.concatenate([bv, bo]).reshape(1, 2 * P)

    maskT = adj.T.astype(f32)  # [m, n]
    maskL = maskT.reshape(MC, P, NH, NHF).transpose(1, 0, 2, 3).reshape(P, -1)

    shared = {
        "cw": cw_b,
        "cbf": cbf_b,
        "br": br.astype(_BF16),
        "maskL": maskL.astype(_BF16),
    }
    in_maps = []
    for b in range(B):
        m = dict(shared)
        m["xqT"] = np.ascontiguousarray(query[b].T).astype(_BF16)
        m["xkT"] = np.ascontiguousarray(key[b].T).astype(_BF16)
        m["xvT"] = np.ascontiguousarray(value[b].T).astype(_BF16)
        in_maps.append(m)
    return in_maps


def kernel(**inputs):
    if "nc" not in _CACHE:
        _CACHE["nc"] = _build_nc()
    nc = _CACHE["nc"]

    from concourse.bass_utils import run_bass_kernel_spmd

    in_maps = _host_prep(**inputs)
    res = run_bass_kernel_spmd(nc, in_maps, core_ids=list(range(NCORES)))
    out = np.stack([res.results[c]["out"] for c in range(NCORES)], axis=0)
    return out.astype(np.float32)


# revision 7
# speedup vs baseline: 1.0315x; 1.0121x over previous
"""Trainium2 Bass kernel v2 for masked dual-softmax attention.

Reference (per batch b, head h; dh=16, H=8, N=1024, D=128):
  q = query @ Wq + bq ; k = key @ Wk + bk ; v = value @ Wv + bv
  S = q_h k_h^T / sqrt(dh)
  attn = 0.5*(softmax(S) + softmax(S masked by adj))
  out = concat_h(attn @ v_h) @ Wo + bo

Sharding: data-parallel over batch, one batch element per core (8 cores).

Device algorithm (S^T layout: m=key idx on partitions, n=query idx free).
Processing runs in 8 groups of (n-half, head-pair); per group 8 m-chunks of
S^T [128,1024] (2 heads x 512 n):
  - head-permuted projection weight tiles put head 4t+j's dh=16 rows on SBUF
    partition quadrant 32j of tile t, so score matmuls pack 2 heads per PSUM
    chunk via tile_position; q/k bias adds are fused into the mandatory
    PSUM->SBUF copies (split ACT/DVE, staged through spare 1-bank PSUM slots
    so the score-chunk bank rotation is never blocked at the start)
  - exp: 7 chunks/group on ACT (the throughput bottleneck), 1 chunk on DVE
    via the bitcast fast-exp exp(x)~=bf16_bits(uint16(x*A+B)) processed as
    two 512-col halves through a dedicated PSUM bank, which keeps the ACT
    exp chain fully decoupled (zero mid-kernel ACT gaps)
  - em = eg*mask on DVE (bf16 2x mode)
  - attn@V transposed: og[n, 17] = sum_mc eg_chunk^T @ [1|v_h]; the 17-wide
    moving operand costs the PE 17 rows instead of 512, and the ones column
    yields the softmax denominators in the same accumulator. 16 sequential
    accumulators per group (the PSUM zero-region rule forbids interleaving
    accumulation groups within one bank); emitted interleaved with the next
    group's score matmuls to keep the PE fed
  - normalize on DVE: reciprocal_approx_fast on the strided denominator
    column + one fused strided multiply into attn[n, k] tiles
  - [n,k] -> [k,n] via PE identity-matmul transposes striped across freed
    PSUM banks + batched per-bank DVE copies
  - output projection: out[n,d] = catT_g^T (0.5 Wo) + catT_m^T (0.5 Wo) + bo
  - drain is pipelined: the last group's accumulators split across both og
    banks so norm/transpose/projection/store of the first n-half overlap
    the second half's accumulation
"""

import sys

if "/opt/trn_rl_repo" not in sys.path:
    sys.path.insert(0, "/opt/trn_rl_repo")

import numpy as np
import ml_dtypes
from contextlib import ExitStack

B, N, D, H, DH = 8, 1024, 128, 8, 16
NCORES = 8
P = 128
NH = 2          # n halves of 512
NHF = N // NH   # 512
MC = 8          # m chunks of 128
G2 = 4          # head groups of 2
NC4 = 4         # n chunks of 128 per half
NG = NH * G2    # 8 groups

_BF16 = ml_dtypes.bfloat16
_CACHE = {}

# exp schedule: "act" (native exp) or "dve" (bitcast fast-exp; DVE only --
# GPSIMD cannot read PSUM). mask schedule: "dve" or "pool".
EXP_SCHEDULE = {(g, mc): "act" for g in range(NG) for mc in range(MC)}
for _g in range(NG):
    EXP_SCHEDULE[(_g, 0)] = "dve"

# fast-exp: exp(x) ~= bf16_bits( uint16( x*A16 + B16 ) )
# (top 16 bits of the classic int32 bitcast trick; uint16 write rounds)
FE_A16 = float(2 ** 23 / np.log(2.0) / 65536.0)
FE_B16 = float((127 * 2 ** 23 - 366000.0) / 65536.0)


def _build_nc():
    import concourse.tile as tile
    import concourse.mybir as mybir
    from concourse import bacc

    bf16 = mybir.dt.bfloat16
    f32 = mybir.dt.float32
    f32r = mybir.dt.float32r
    u16 = mybir.dt.uint16
    Exp = mybir.ActivationFunctionType.Exp
    Alu = mybir.AluOpType

    nc = bacc.Bacc("TRN2", target_bir_lowering=False, debug=False,
                   num_devices=NCORES)

    # ---- DRAM I/O -------------------------------------------------------
    # cw: wqa|wqb|wka|wkb (4x128 cols) + bqa|bqb|bka|bkb (4 f32 cols
    # stored as 8 bf16 columns bit-cast)
    cw_d = nc.dram_tensor("cw", [P, 4 * P + 8], bf16, kind="ExternalInput")
    # cbf: wv|wog|identity (3x128 cols)
    cbf_d = nc.dram_tensor("cbf", [P, 3 * P], bf16, kind="ExternalInput")
    # br: bvr|bor rows
    br_d = nc.dram_tensor("br", [1, 2 * P], bf16, kind="ExternalInput")
    xq_d = nc.dram_tensor("xqT", [P, N], bf16, kind="ExternalInput")
    xk_d = nc.dram_tensor("xkT", [P, N], bf16, kind="ExternalInput")
    xv_d = nc.dram_tensor("xvT", [P, N], bf16, kind="ExternalInput")
    mask_d = nc.dram_tensor("maskL", [P, MC * NH * NHF], bf16, kind="ExternalInput")
    out_d = nc.dram_tensor("out", [N, D], f32, kind="ExternalOutput")

    with tile.TileContext(nc) as tc, ExitStack() as ctx:
        const = ctx.enter_context(tc.tile_pool(name="const", bufs=1))
        xpool = ctx.enter_context(tc.tile_pool(name="x", bufs=1))
        qkpool = ctx.enter_context(tc.tile_pool(name="qk", bufs=1))
        egp = ctx.enter_context(tc.tile_pool(name="eg", bufs=3))
        emp = ctx.enter_context(tc.tile_pool(name="em", bufs=3))
        recp = ctx.enter_context(tc.tile_pool(name="rec", bufs=2))
        atp = ctx.enter_context(tc.tile_pool(name="attn", bufs=2))
        ctp = ctx.enter_context(tc.tile_pool(name="catT", bufs=2))
        osb = ctx.enter_context(tc.tile_pool(name="osb", bufs=2))
        # PSUM: s4 2 banks x2, og 1 bank x2, outp 1 bank, fexp 1 bank = 8
        s4p = ctx.enter_context(tc.tile_pool(name="s4", bufs=2, space="PSUM"))
        ogp = ctx.enter_context(tc.tile_pool(name="og", bufs=2, space="PSUM"))
        outp = ctx.enter_context(tc.tile_pool(name="outp", bufs=1, space="PSUM"))
        fxp = ctx.enter_context(tc.tile_pool(name="fxp", bufs=1, space="PSUM"))

        # ---- constants / inputs (order = DMA issue order) ----------------
        cw = const.tile([P, 4 * P + 8], bf16, tag="cw")
        xq = xpool.tile([P, N], bf16, tag="xq")
        xk = xpool.tile([P, N], bf16, tag="xk")
        xv = xpool.tile([P, N], bf16, tag="xv")
        nc.sync.dma_start(xk[:], xk_d.ap())
        nc.sync.dma_start(cw[:], cw_d.ap())
        nc.sync.dma_start(xq[:], xq_d.ap())
        nc.sync.dma_start(xv[:], xv_d.ap())
        cbf = const.tile([P, 3 * P], bf16, tag="cbf")
        nc.sync.dma_start(cbf[:], cbf_d.ap())
        br = const.tile([1, 2 * P], bf16, tag="br")
        nc.sync.dma_start(br[:], br_d.ap())
        mask_sb = const.tile([P, MC, NH, NHF], bf16, tag="mask")
        mask_dr = mask_d.ap().rearrange("p (a b f) -> p a b f", a=MC, b=NH)
        for _mc in range(MC):
            nc.sync.dma_start(mask_sb[:, _mc, :, :], mask_dr[:, _mc, :, :])

        wqa, wqb = cw[:, 0:P], cw[:, P:2 * P]
        wka, wkb = cw[:, 2 * P:3 * P], cw[:, 3 * P:4 * P]
        bias32 = cw[:, 4 * P:4 * P + 8].bitcast(f32)
        bias_col = {"qa": bias32[:, 0:1], "qb": bias32[:, 1:2],
                    "ka": bias32[:, 2:3], "kb": bias32[:, 3:4]}
        wv, wog, ident = cbf[:, 0:P], cbf[:, P:2 * P], cbf[:, 2 * P:3 * P]
        bvr, bor = br[:, 0:P], br[:, P:2 * P]

        ones1 = const.tile([1, P], bf16, tag="ones1")
        nc.vector.memset(ones1[:], 1.0)
        # tiny dummy activation: anchors the one-time ACT table load at t~0
        # (otherwise it attaches to the first real activation mid-head)
        dummy = const.tile([1, 1], bf16, tag="dummy")
        nc.scalar.activation(dummy[:], ones1[0:1, 0:1],
                             mybir.ActivationFunctionType.Exp)

        # ---- PE warm-up: ramp the p-state while input DMAs land ----------
        warm = ogp.tile([P, 512], f32, tag="og", name="warm")
        for _w in range(24):
            nc.tensor.matmul(warm[:, 0:P], ones1[:], ones1[:],
                             start=True, stop=True)

        # ---- q/k projections (head-permuted quadrant packing) ------------
        # qa/ka use the two 2-bank s4 slots (freed ASAP for the score
        # chunks); qb/kb go through the 1-bank fxp / og slots in halves so
        # they never block the s4 rotation.
        qk_tiles = {}
        for name in ("qa", "ka", "qb", "kb"):
            qk_tiles[name] = qkpool.tile([P, N], f32r, tag=name, name=name)
        # qa: n-half 1 through the fxp bank, copied on ACT (fills early ACT
        # idle; covers all nh=0 score chunks); n-half 2 via og bank on DVE
        qah1 = fxp.tile([P, NHF], f32, tag="sf", name="qah1")
        nc.tensor.matmul(qah1[:], wqa, xq[:, 0:NHF], start=True, stop=True)
        nc.scalar.add(qk_tiles["qa"][:, 0:NHF], qah1[:], bias_col["qa"])
        qah2 = ogp.tile([P, 512], f32, tag="og", name="qah2")
        nc.tensor.matmul(qah2[:], wqa, xq[:, NHF:N], start=True, stop=True)
        nc.scalar.add(qk_tiles["qa"][:, NHF:N], qah2[:], bias_col["qa"])
        # ka: s4-pool psum; m-cols 0:256 copied first (unblocks s4(0,1))
        ps_ka = s4p.tile([P, N], f32, tag="s4", name="proj_ka")
        for s in range(NH):
            nc.tensor.matmul(ps_ka[:, s * NHF:(s + 1) * NHF], wka,
                             xk[:, s * NHF:(s + 1) * NHF],
                             start=True, stop=True)
        nc.vector.tensor_scalar_add(qk_tiles["ka"][:, 0:256],
                                    ps_ka[:, 0:256], bias_col["ka"])
        nc.vector.tensor_scalar_add(qk_tiles["ka"][:, 256:N],
                                    ps_ka[:, 256:N], bias_col["ka"])
        def emit_b_projections():
            # qb: half 1 in the fxp bank, half 2 in an og bank
            qbh1 = fxp.tile([P, NHF], f32, tag="sf", name="qbh1")
            nc.tensor.matmul(qbh1[:], wqb, xq[:, 0:NHF], start=True, stop=True)
            nc.vector.tensor_scalar_add(qk_tiles["qb"][:, 0:NHF], qbh1[:],
                                        bias_col["qb"])
            qbh2 = ogp.tile([P, 512], f32, tag="og", name="qbh2")
            nc.tensor.matmul(qbh2[:], wqb, xq[:, NHF:N], start=True, stop=True)
            nc.vector.tensor_scalar_add(qk_tiles["qb"][:, NHF:N], qbh2[:],
                                        bias_col["qb"])
            # kb: two og-bank halves
            kbh1 = ogp.tile([P, 512], f32, tag="og", name="kbh1")
            nc.tensor.matmul(kbh1[:], wkb, xk[:, 0:NHF], start=True, stop=True)
            nc.vector.tensor_scalar_add(qk_tiles["kb"][:, 0:NHF], kbh1[:],
                                        bias_col["kb"])
            kbh2 = ogp.tile([P, 512], f32, tag="og", name="kbh2")
            nc.tensor.matmul(kbh2[:], wkb, xk[:, NHF:N], start=True, stop=True)
            nc.vector.tensor_scalar_add(qk_tiles["kb"][:, NHF:N], kbh2[:],
                                        bias_col["kb"])

        # ---- v projection: natural layout [m, dim], augmented [1|v] ------
        # vaug[p, mc, h, 0] = 1 ; vaug[p, mc, h, 1+c] = V[mc*128+p, 16h+c]
        vaug = qkpool.tile([P, MC, H, 17], bf16, tag="vaug")
        nc.vector.memset(vaug[:, :, :, 0], 1.0)

        vaug_copies = []

        def emit_vp():
            for half in range(2):
                vp = outp.tile([P, 4, P], f32, tag="outp", name=f"vps{half}")
                for mc4 in range(4):
                    mc = half * 4 + mc4
                    sl = vp[:, mc4, :]
                    nc.tensor.matmul(sl, ones1[:], bvr,
                                     start=True, stop=False)
                    nc.tensor.matmul(sl, xv[:, mc * P:(mc + 1) * P], wv,
                                     start=False, stop=True)
                vaug_copies.append(vp[:])

        # ---- main loop --------------------------------------------------
        groups = [(nh, g2) for nh in range(NH) for g2 in range(G2)]
        state = {}

        def emit_og_accs(gi, accs, og=None, wbase=0):
            """Emit the given accumulator indices (0..15) of group gi."""
            nh, g2 = groups[gi]
            st = state[gi]
            og = og if og is not None else st["og"]
            for acc in accs:
                nc4, r = divmod(acc, 4)
                a, i = divmod(r, 2)
                tiles = st["egs"] if a == 0 else st["ems"]
                w = (acc - wbase) * 17
                for mc in range(MC):
                    nc.tensor.matmul(
                        og[:, w:w + 17],
                        tiles[mc][:, i * NHF + nc4 * P:i * NHF + (nc4 + 1) * P],
                        vaug[:, mc, 2 * g2 + i, :],
                        start=(mc == 0), stop=(mc == MC - 1))

        def emit_norm(gi, og=None, nc0=0, ncn=NC4, suffix=""):
            """Normalize accumulators for nc4 range [nc0, nc0+ncn) held in
            `og` (windows start at (nc4-nc0)*68 cols)."""
            nh, g2 = groups[gi]
            st = state[gi]
            og = og if og is not None else st["og"]
            nw = ncn * 4
            ogv = og[:, 0:17 * nw].rearrange("p (w c) -> p w c", c=17)
            rec = recp.tile([P, 16], f32, tag="rec", name=f"rec_{gi}{suffix}")
            nc.vector.reciprocal_approx_fast(rec[:, 0:nw], ogv[:, :, 0])
            attn = state.get(("attn", nh))
            if attn is None:
                attn = atp.tile([P, NC4, 2, P], bf16, tag="attn",
                                name=f"attn_{nh}")
                state[("attn", nh)] = attn
            in0 = og[:, 0:17 * nw].rearrange(
                "p (nc a i c) -> p nc a i c", nc=ncn, a=2, i=2)[:, :, :, :, 1:17]
            in1 = rec[:, 0:nw].rearrange(
                "p (nc a i) -> p nc a i", nc=ncn, a=2)[:, :, :, :, None]
            out = attn[:, nc0:nc0 + ncn, :, 32 * g2:32 * g2 + 32].rearrange(
                "p nc a (i c) -> p nc a i c", i=2)
            nc.vector.tensor_mul(out, in0,
                                 in1.broadcast_to([P, ncn, 2, 2, 16]))

        def setup_tr(nh):
            # window idx 0-3 -> bank-0 cols, idx 4-7 -> bank-1 cols, so one
            # strided copy drains each 4-window bank
            if nh == NH - 1:
                tr = s4p.tile([P, N], f32, tag="s4", name="tr_final")
                t16 = tr[:].bitcast(bf16).rearrange("p (i c) -> p i c", c=P)
            else:
                tr = outp.tile([P, 4, P], f32, tag="outp", name=f"tr_{nh}")
                t16 = tr[:].bitcast(bf16).rearrange(
                    "p a (j c) -> p (a j) c", j=2)
            win = lambda idx: t16[:, (idx // 4) * (8 if nh == NH - 1 else 4)
                                  + idx % 4, :]
            state[("trwin", nh)] = win
            state[("trt16", nh)] = t16
            ctf = ctp.tile([P, 8, P], bf16, tag="ctf", name=f"ctf_{nh}")
            state[("ctf", nh)] = ctf

        def emit_transposes(nh):
            setup_tr(nh)
            emit_tr_sub(nh, range(NC4))

        def emit_tr_sub(nh, nc4s):
            attn = state[("attn", nh)]
            win = state[("trwin", nh)]
            for nc4 in nc4s:
                for a in range(2):
                    nc.tensor.transpose(win(nc4 * 2 + a),
                                        attn[:, nc4, a, :], ident)
            emit_ct_copies(nh, nc4s)

        def emit_ct_copies(nh, nc4s):
            # one batched copy per 4-window group (= per bank)
            t16 = state[("trt16", nh)]
            ctf = state[("ctf", nh)]
            cts = state.setdefault(("cts", nh), {})
            idxs = sorted(nc4 * 2 + a for nc4 in nc4s for a in range(2))
            base = 8 if nh == NH - 1 else 4
            for grp0 in (0, 4):
                grp = [i for i in idxs if grp0 <= i < grp0 + 4]
                if not grp:
                    continue
                lo, hi = min(grp), max(grp)
                src_ap = t16[:, (grp0 // 4) * base + lo % 4:
                             (grp0 // 4) * base + hi % 4 + 1, :]
                nc.vector.tensor_copy(ctf[:, lo:hi + 1, :], src_ap)
            for nc4 in nc4s:
                cts[nc4] = [ctf[:, nc4 * 2, :], ctf[:, nc4 * 2 + 1, :]]

        def emit_proj(nh, nc4s=range(NC4), part=0):
            cts = state[("cts", nh)]
            if part == 0 or ("op", nh) not in state:
                state[("op", nh)] = outp.tile([P, NC4, P], f32, tag="outp",
                                              name=f"op_{nh}")
                state[("ob", nh)] = osb.tile([P, NC4, P], f32, tag="osb",
                                             name=f"ob_{nh}")
            op = state[("op", nh)]
            ob = state[("ob", nh)]
            for nc4 in nc4s:
                sl = op[:, nc4, :]
                nc.tensor.matmul(sl, ones1[:], bor, start=True, stop=False)
                nc.tensor.matmul(sl, cts[nc4][0][:], wog,
                                 start=False, stop=False)
                nc.tensor.matmul(sl, cts[nc4][1][:], wog,
                                 start=False, stop=True)
            lo, hi = min(nc4s), max(nc4s) + 1
            if nh == NH - 1 and part == 0:
                nc.scalar.copy(ob[:, lo:hi, :], op[:, lo:hi, :])
            else:
                nc.vector.tensor_copy(ob[:, lo:hi, :], op[:, lo:hi, :])
            dst = out_d.ap().rearrange("(x c p) d -> x p c d", x=NH, c=NC4)[nh]
            nc.sync.dma_start(dst[:, lo:hi, :], ob[:, lo:hi, :])

        # accumulator emission order: eg-based accs first (their inputs are
        # ready one em-latency earlier at the group boundary)
        EG_ACCS = [a for a in range(16) if (a % 4) // 2 == 0]
        EM_ACCS = [a for a in range(16) if (a % 4) // 2 == 1]

        def emit_s4(gi, mc):
            nh, g2 = groups[gi]
            s4 = s4p.tile([P, N], f32, tag="s4", name=f"s4_{gi}_{mc}")
            t = "a" if g2 < 2 else "b"
            qT = qk_tiles["q" + t]
            kT = qk_tiles["k" + t]
            q0 = (2 * g2) % 4
            for i in range(2):
                qq = 32 * (q0 + i)
                nc.tensor.matmul(
                    s4[:, i * NHF:(i + 1) * NHF],
                    kT[qq:qq + 16, mc * P:(mc + 1) * P],
                    qT[qq:qq + 16, nh * NHF:(nh + 1) * NHF],
                    start=True, stop=True, tile_position=(qq, 0))
            return s4

        def emit_sf(gi, mc, i):
            """Score matmul for head-half i of chunk mc via the fxp bank."""
            nh, g2 = groups[gi]
            sf = fxp.tile([P, NHF], f32, tag="sf", name=f"sf_{gi}_{mc}_{i}")
            q0 = (2 * g2) % 4
            qq = 32 * (q0 + i)
            t = "a" if g2 < 2 else "b"
            nc.tensor.matmul(
                sf[:], qk_tiles["k" + t][qq:qq + 16, mc * P:(mc + 1) * P],
                qk_tiles["q" + t][qq:qq + 16, nh * NHF:(nh + 1) * NHF],
                start=True, stop=True, tile_position=(qq, 0))
            return sf

        def emit_exp(gi, mc, s4):
            eg = egp.tile([P, N], bf16, tag=f"eg{mc}", name=f"eg_{gi}_{mc}")
            nc.scalar.activation(eg[:], s4[:], Exp)
            return eg

        def emit_em(gi, mc, eg):
            nh, g2 = groups[gi]
            em = emp.tile([P, N], bf16, tag=f"em{mc}", name=f"em_{gi}_{mc}")
            msk = mask_sb[:, mc, nh, :]
            nc.vector.tensor_mul(
                em[:].rearrange("p (i f) -> p i f", i=2),
                eg[:].rearrange("p (i f) -> p i f", i=2),
                msk[:, None, :].broadcast_to([P, 2, NHF]))
            return em

        for gi in range(NG):
            nh, g2 = groups[gi]
            if gi > 0:
                state[gi - 1]["og"] = ogp.tile([P, 512], f32, tag="og",
                                               name=f"og_{gi - 1}")
            egs = [None] * MC
            ems = [None] * MC
            # fexp chunk (mc=0): half 0 scores + DVE bitcast exp
            eg0 = egp.tile([P, N], bf16, tag="eg0", name=f"eg_{gi}_0")
            sf0 = emit_sf(gi, 0, 0)
            nc.vector.tensor_scalar(eg0[:, 0:NHF].bitcast(u16), sf0[:],
                                    FE_A16, FE_B16,
                                    op0=Alu.mult, op1=Alu.add)
            s4_1 = emit_s4(gi, 1)
            egs[1] = emit_exp(gi, 1, s4_1)
            s4_2 = emit_s4(gi, 2)
            egs[2] = emit_exp(gi, 2, s4_2)
            sf1 = emit_sf(gi, 0, 1)
            nc.vector.tensor_scalar(eg0[:, NHF:N].bitcast(u16), sf1[:],
                                    FE_A16, FE_B16,
                                    op0=Alu.mult, op1=Alu.add)
            egs[0] = eg0
            ems[0] = emit_em(gi, 0, eg0)
            # first half of previous group's attn@V accumulators
            if gi > 0:
                emit_og_accs(gi - 1, EG_ACCS)
            ems[1] = emit_em(gi, 1, egs[1])
            ems[2] = emit_em(gi, 2, egs[2])

            if gi == 0:
                emit_b_projections()
                emit_vp()
            s4_3 = emit_s4(gi, 3)
            egs[3] = emit_exp(gi, 3, s4_3)
            ems[3] = emit_em(gi, 3, egs[3])
            if gi > 0:
                emit_og_accs(gi - 1, EM_ACCS)
                emit_norm(gi - 1)
            if gi == 5:
                emit_transposes(0)
            if gi == 7:
                emit_proj(0)
            for mc in range(4, MC):
                s4 = emit_s4(gi, mc)
                egs[mc] = emit_exp(gi, mc, s4)
                ems[mc] = emit_em(gi, mc, egs[mc])
                if gi == 0 and mc in (4, 6):
                    half = (mc - 4) // 2
                    vp4 = vaug_copies[half]
                    nc.vector.tensor_copy(
                        vaug[:, half * 4:half * 4 + 4, :, 1:17],
                        vp4.rearrange("p c (h d) -> p c h d", h=H))
            state[gi] = {"egs": egs, "ems": ems}

        # drain: pipelined across the two og banks (nc4 halves)
        gi = NG - 1
        og7a = ogp.tile([P, 512], f32, tag="og", name="og7a")
        og7b = ogp.tile([P, 512], f32, tag="og", name="og7b")
        emit_og_accs(gi, range(0, 8), og=og7a, wbase=0)      # nc4 0-1
        emit_og_accs(gi, range(8, 16), og=og7b, wbase=8)     # nc4 2-3
        emit_norm(gi, og=og7a, nc0=0, ncn=2, suffix="a")
        setup_tr(NH - 1)
        emit_tr_sub(NH - 1, (0, 1))
        emit_norm(gi, og=og7b, nc0=2, ncn=2, suffix="b")
        emit_proj(NH - 1, nc4s=(0, 1), part=0)
        emit_tr_sub(NH - 1, (2, 3))
        emit_proj(NH - 1, nc4s=(2, 3), part=1)

    nc.compile()
    return nc


def _host_prep(query, key, value, adj_mask, Wq, bq, Wk, bk, Wv, bv, Wo, bo):
    """Per-core input maps (host-side layout transforms only)."""
    f32 = np.float32
    query = np.asarray(query, f32)
    key = np.asarray(key, f32)
    value = np.asarray(value, f32)
    Wq = np.asarray(Wq, f32); Wk = np.asarray(Wk, f32)
    Wv = np.asarray(Wv, f32); Wo = np.asarray(Wo, f32)
    bq = np.asarray(bq, f32); bk = np.asarray(bk, f32)
    bv = np.asarray(bv, f32); bo = np.asarray(bo, f32)
    adj = np.asarray(adj_mask)

    scale = 1.0 / np.sqrt(np.float32(DH))

    def pack_w(Wm):
        out = []
        for t in range(2):
            wt = np.zeros((P, P), f32)
            for j in range(4):
                h = 4 * t + j
                wt[:, 32 * j:32 * j + 16] = Wm[:, DH * h:DH * (h + 1)]
            out.append(wt)
        return out

    wqa, wqb = [w * scale for w in pack_w(Wq)]
    wka, wkb = pack_w(Wk)

    def pack_b(bvec, s):
        cols = []
        for t in range(2):
            col = np.zeros((P,), f32)
            for j in range(4):
                h = 4 * t + j
                col[32 * j:32 * j + 16] = bvec[DH * h:DH * (h + 1)] * s
            cols.append(col)
        return cols

    bqa, bqb = pack_b(bq, scale)
    bka, bkb = pack_b(bk, 1.0)

    cw = np.zeros((P, 4 * P), f32)
    cw[:, 0:P] = wqa; cw[:, P:2 * P] = wqb
    cw[:, 2 * P:3 * P] = wka; cw[:, 3 * P:4 * P] = wkb
    bias4 = np.stack([bqa, bqb, bka, bkb], axis=1).astype(f32)  # [P, 4]
    cw_b = np.concatenate([cw.astype(_BF16),
                           bias4.view(_BF16).reshape(P, 8)], axis=1)

    cbf = np.zeros((P, 3 * P), f32)
    cbf[:, 0:P] = Wv
    cbf[:, P:2 * P] = 0.5 * Wo
    cbf[:, 2 * P:3 * P] = np.eye(P, dtype=f32)
    cbf_b = cbf.astype(_BF16)

    br = np.concatenate([bv, bo]).reshape(1, 2 * P)

    maskT = adj.T.astype(f32)  # [m, n]
    maskL = maskT.reshape(MC, P, NH, NHF).transpose(1, 0, 2, 3).reshape(P, -1)

    shared = {
        "cw": cw_b,
        "cbf": cbf_b,
        "br": br.astype(_BF16),
        "maskL": maskL.astype(_BF16),
    }
    in_maps = []
    for b in range(B):
        m = dict(shared)
        m["xqT"] = np.ascontiguousarray(query[b].T).astype(_BF16)
        m["xkT"] = np.ascontiguousarray(key[b].T).astype(_BF16)
        m["xvT"] = np.ascontiguousarray(value[b].T).astype(_BF16)
        in_maps.append(m)
    return in_maps


def kernel(**inputs):
    if "nc" not in _CACHE:
        _CACHE["nc"] = _build_nc()
    nc = _CACHE["nc"]

    from concourse.bass_utils import run_bass_kernel_spmd

    in_maps = _host_prep(**inputs)
    res = run_bass_kernel_spmd(nc, in_maps, core_ids=list(range(NCORES)))
    out = np.stack([res.results[c]["out"] for c in range(NCORES)], axis=0)
    return out.astype(np.float32)


# revision 8
# speedup vs baseline: 1.0329x; 1.0013x over previous
"""Trainium2 Bass kernel v2 for masked dual-softmax attention.

Reference (per batch b, head h; dh=16, H=8, N=1024, D=128):
  q = query @ Wq + bq ; k = key @ Wk + bk ; v = value @ Wv + bv
  S = q_h k_h^T / sqrt(dh)
  attn = 0.5*(softmax(S) + softmax(S masked by adj))
  out = concat_h(attn @ v_h) @ Wo + bo

Sharding: data-parallel over batch, one batch element per core (8 cores).

Device algorithm (S^T layout: m=key idx on partitions, n=query idx free).
Processing runs in 8 groups of (n-half, head-pair); per group 8 m-chunks of
S^T [128,1024] (2 heads x 512 n):
  - head-permuted projection weight tiles put head 4t+j's dh=16 rows on SBUF
    partition quadrant 32j of tile t, so score matmuls pack 2 heads per PSUM
    chunk via tile_position; q/k bias adds are fused into the mandatory
    PSUM->SBUF copies (split ACT/DVE, staged through spare 1-bank PSUM slots
    so the score-chunk bank rotation is never blocked at the start)
  - exp: 7 chunks/group on ACT (the throughput bottleneck), 1 chunk on DVE
    via the bitcast fast-exp exp(x)~=bf16_bits(uint16(x*A+B)) processed as
    two 512-col halves through a dedicated PSUM bank, which keeps the ACT
    exp chain fully decoupled (zero mid-kernel ACT gaps)
  - em = eg*mask on DVE (bf16 2x mode)
  - attn@V transposed: og[n, 17] = sum_mc eg_chunk^T @ [1|v_h]; the 17-wide
    moving operand costs the PE 17 rows instead of 512, and the ones column
    yields the softmax denominators in the same accumulator. 16 sequential
    accumulators per group (the PSUM zero-region rule forbids interleaving
    accumulation groups within one bank); emitted interleaved with the next
    group's score matmuls to keep the PE fed
  - normalize on DVE: reciprocal_approx_fast on the strided denominator
    column + one fused strided multiply into attn[n, k] tiles
  - [n,k] -> [k,n] via PE identity-matmul transposes striped across freed
    PSUM banks + batched per-bank DVE copies
  - output projection: out[n,d] = catT_g^T (0.5 Wo) + catT_m^T (0.5 Wo) + bo
  - drain is pipelined: the last group's accumulators split across both og
    banks so norm/transpose/projection/store of the first n-half overlap
    the second half's accumulation
"""

import sys

if "/opt/trn_rl_repo" not in sys.path:
    sys.path.insert(0, "/opt/trn_rl_repo")

import numpy as np
import ml_dtypes
from contextlib import ExitStack

B, N, D, H, DH = 8, 1024, 128, 8, 16
NCORES = 8
P = 128
NH = 2          # n halves of 512
NHF = N // NH   # 512
MC = 8          # m chunks of 128
G2 = 4          # head groups of 2
NC4 = 4         # n chunks of 128 per half
NG = NH * G2    # 8 groups

_BF16 = ml_dtypes.bfloat16
_CACHE = {}

# exp schedule: "act" (native exp) or "dve" (bitcast fast-exp; DVE only --
# GPSIMD cannot read PSUM). mask schedule: "dve" or "pool".
EXP_SCHEDULE = {(g, mc): "act" for g in range(NG) for mc in range(MC)}
for _g in range(NG):
    EXP_SCHEDULE[(_g, 0)] = "dve"

# fast-exp: exp(x) ~= bf16_bits( uint16( x*A16 + B16 ) )
# (top 16 bits of the classic int32 bitcast trick; uint16 write rounds)
FE_A16 = float(2 ** 23 / np.log(2.0) / 65536.0)
FE_B16 = float((127 * 2 ** 23 - 366000.0) / 65536.0)


def _build_nc():
    import concourse.tile as tile
    import concourse.mybir as mybir
    from concourse import bacc

    bf16 = mybir.dt.bfloat16
    f32 = mybir.dt.float32
    f32r = mybir.dt.float32r
    u16 = mybir.dt.uint16
    Exp = mybir.ActivationFunctionType.Exp
    Alu = mybir.AluOpType

    nc = bacc.Bacc("TRN2", target_bir_lowering=False, debug=False,
                   num_devices=NCORES)

    # ---- DRAM I/O -------------------------------------------------------
    # cw: wqa|wqb|wka|wkb (4x128 cols) + bqa|bqb|bka|bkb (4 f32 cols
    # stored as 8 bf16 columns bit-cast)
    cw_d = nc.dram_tensor("cw", [P, 4 * P + 8], bf16, kind="ExternalInput")
    # cbf: wv|wog|identity (3x128 cols)
    cbf_d = nc.dram_tensor("cbf", [P, 3 * P], bf16, kind="ExternalInput")
    # br: bvr|bor rows
    br_d = nc.dram_tensor("br", [1, 2 * P], bf16, kind="ExternalInput")
    xq_d = nc.dram_tensor("xqT", [P, N], bf16, kind="ExternalInput")
    xk_d = nc.dram_tensor("xkT", [P, N], bf16, kind="ExternalInput")
    xv_d = nc.dram_tensor("xvT", [P, N], bf16, kind="ExternalInput")
    mask_d = nc.dram_tensor("maskL", [P, MC * NH * NHF], bf16, kind="ExternalInput")
    out_d = nc.dram_tensor("out", [N, D], f32, kind="ExternalOutput")

    with tile.TileContext(nc) as tc, ExitStack() as ctx:
        const = ctx.enter_context(tc.tile_pool(name="const", bufs=1))
        xpool = ctx.enter_context(tc.tile_pool(name="x", bufs=1))
        qkpool = ctx.enter_context(tc.tile_pool(name="qk", bufs=1))
        egp = ctx.enter_context(tc.tile_pool(name="eg", bufs=3))
        emp = ctx.enter_context(tc.tile_pool(name="em", bufs=3))
        recp = ctx.enter_context(tc.tile_pool(name="rec", bufs=2))
        atp = ctx.enter_context(tc.tile_pool(name="attn", bufs=2))
        ctp = ctx.enter_context(tc.tile_pool(name="catT", bufs=2))
        osb = ctx.enter_context(tc.tile_pool(name="osb", bufs=2))
        # PSUM: s4 2 banks x2, og 1 bank x2, outp 1 bank, fexp 1 bank = 8
        s4p = ctx.enter_context(tc.tile_pool(name="s4", bufs=2, space="PSUM"))
        ogp = ctx.enter_context(tc.tile_pool(name="og", bufs=2, space="PSUM"))
        outp = ctx.enter_context(tc.tile_pool(name="outp", bufs=1, space="PSUM"))
        fxp = ctx.enter_context(tc.tile_pool(name="fxp", bufs=1, space="PSUM"))

        # ---- constants / inputs (order = DMA issue order) ----------------
        cw = const.tile([P, 4 * P + 8], bf16, tag="cw")
        xq = xpool.tile([P, N], bf16, tag="xq")
        xk = xpool.tile([P, N], bf16, tag="xk")
        xv = xpool.tile([P, N], bf16, tag="xv")
        nc.sync.dma_start(xk[:], xk_d.ap())
        nc.sync.dma_start(cw[:], cw_d.ap())
        nc.sync.dma_start(xq[:], xq_d.ap())
        nc.sync.dma_start(xv[:], xv_d.ap())
        cbf = const.tile([P, 3 * P], bf16, tag="cbf")
        nc.sync.dma_start(cbf[:], cbf_d.ap())
        br = const.tile([1, 2 * P], bf16, tag="br")
        nc.sync.dma_start(br[:], br_d.ap())
        mask_sb = const.tile([P, MC, NH, NHF], bf16, tag="mask")
        mask_dr = mask_d.ap().rearrange("p (a b f) -> p a b f", a=MC, b=NH)
        for _mc in range(MC):
            nc.sync.dma_start(mask_sb[:, _mc, :, :], mask_dr[:, _mc, :, :])

        wqa, wqb = cw[:, 0:P], cw[:, P:2 * P]
        wka, wkb = cw[:, 2 * P:3 * P], cw[:, 3 * P:4 * P]
        bias32 = cw[:, 4 * P:4 * P + 8].bitcast(f32)
        bias_col = {"qa": bias32[:, 0:1], "qb": bias32[:, 1:2],
                    "ka": bias32[:, 2:3], "kb": bias32[:, 3:4]}
        wv, wog, ident = cbf[:, 0:P], cbf[:, P:2 * P], cbf[:, 2 * P:3 * P]
        bvr, bor = br[:, 0:P], br[:, P:2 * P]

        ones1 = const.tile([1, P], bf16, tag="ones1")
        nc.vector.memset(ones1[:], 1.0)
        # tiny dummy activation: anchors the one-time ACT table load at t~0
        # (otherwise it attaches to the first real activation mid-head)
        dummy = const.tile([1, 1], bf16, tag="dummy")
        nc.scalar.activation(dummy[:], ones1[0:1, 0:1],
                             mybir.ActivationFunctionType.Exp)

        # ---- PE warm-up: ramp the p-state while input DMAs land ----------
        warm = ogp.tile([P, 512], f32, tag="og", name="warm")
        for _w in range(24):
            nc.tensor.matmul(warm[:, 0:P], ones1[:], ones1[:],
                             start=True, stop=True)

        # ---- q/k projections (head-permuted quadrant packing) ------------
        # qa/ka use the two 2-bank s4 slots (freed ASAP for the score
        # chunks); qb/kb go through the 1-bank fxp / og slots in halves so
        # they never block the s4 rotation.
        qk_tiles = {}
        for name in ("qa", "ka", "qb", "kb"):
            qk_tiles[name] = qkpool.tile([P, N], f32r, tag=name, name=name)
        # qa: n-half 1 through the fxp bank, copied on ACT (fills early ACT
        # idle; covers all nh=0 score chunks); n-half 2 via og bank on DVE
        qah1 = fxp.tile([P, NHF], f32, tag="sf", name="qah1")
        nc.tensor.matmul(qah1[:], wqa, xq[:, 0:NHF], start=True, stop=True)
        nc.scalar.add(qk_tiles["qa"][:, 0:NHF], qah1[:], bias_col["qa"])
        qah2 = ogp.tile([P, 512], f32, tag="og", name="qah2")
        nc.tensor.matmul(qah2[:], wqa, xq[:, NHF:N], start=True, stop=True)
        nc.scalar.add(qk_tiles["qa"][:, NHF:N], qah2[:], bias_col["qa"])
        # ka: s4-pool psum; m-cols 0:256 copied first (unblocks s4(0,1))
        ps_ka = s4p.tile([P, N], f32, tag="s4", name="proj_ka")
        for s in range(NH):
            nc.tensor.matmul(ps_ka[:, s * NHF:(s + 1) * NHF], wka,
                             xk[:, s * NHF:(s + 1) * NHF],
                             start=True, stop=True)
        nc.vector.tensor_scalar_add(qk_tiles["ka"][:, 0:256],
                                    ps_ka[:, 0:256], bias_col["ka"])
        nc.vector.tensor_scalar_add(qk_tiles["ka"][:, 256:N],
                                    ps_ka[:, 256:N], bias_col["ka"])
        def emit_b_projections():
            # qb: half 1 in the fxp bank, half 2 in an og bank
            qbh1 = fxp.tile([P, NHF], f32, tag="sf", name="qbh1")
            nc.tensor.matmul(qbh1[:], wqb, xq[:, 0:NHF], start=True, stop=True)
            nc.vector.tensor_scalar_add(qk_tiles["qb"][:, 0:NHF], qbh1[:],
                                        bias_col["qb"])
            qbh2 = ogp.tile([P, 512], f32, tag="og", name="qbh2")
            nc.tensor.matmul(qbh2[:], wqb, xq[:, NHF:N], start=True, stop=True)
            nc.vector.tensor_scalar_add(qk_tiles["qb"][:, NHF:N], qbh2[:],
                                        bias_col["qb"])
            # kb: two og-bank halves
            kbh1 = ogp.tile([P, 512], f32, tag="og", name="kbh1")
            nc.tensor.matmul(kbh1[:], wkb, xk[:, 0:NHF], start=True, stop=True)
            nc.vector.tensor_scalar_add(qk_tiles["kb"][:, 0:NHF], kbh1[:],
                                        bias_col["kb"])
            kbh2 = ogp.tile([P, 512], f32, tag="og", name="kbh2")
            nc.tensor.matmul(kbh2[:], wkb, xk[:, NHF:N], start=True, stop=True)
            nc.vector.tensor_scalar_add(qk_tiles["kb"][:, NHF:N], kbh2[:],
                                        bias_col["kb"])

        # ---- v projection: natural layout [m, dim], augmented [1|v] ------
        # vaug[p, mc, h, 0] = 1 ; vaug[p, mc, h, 1+c] = V[mc*128+p, 16h+c]
        vaug = qkpool.tile([P, MC, H, 17], bf16, tag="vaug")
        nc.vector.memset(vaug[:, :, :, 0], 1.0)

        vaug_copies = []

        def emit_vp():
            for half in range(2):
                vp = outp.tile([P, 4, P], f32, tag="outp", name=f"vps{half}")
                for mc4 in range(4):
                    mc = half * 4 + mc4
                    sl = vp[:, mc4, :]
                    nc.tensor.matmul(sl, ones1[:], bvr,
                                     start=True, stop=False)
                    nc.tensor.matmul(sl, xv[:, mc * P:(mc + 1) * P], wv,
                                     start=False, stop=True)
                vaug_copies.append(vp[:])

        # ---- main loop --------------------------------------------------
        groups = [(nh, g2) for nh in range(NH) for g2 in range(G2)]
        state = {}

        def emit_og_accs(gi, accs, og=None, wbase=0):
            """Emit the given accumulator indices (0..15) of group gi."""
            nh, g2 = groups[gi]
            st = state[gi]
            og = og if og is not None else st["og"]
            for acc in accs:
                nc4, r = divmod(acc, 4)
                a, i = divmod(r, 2)
                tiles = st["egs"] if a == 0 else st["ems"]
                w = (acc - wbase) * 17
                for mc in range(MC):
                    nc.tensor.matmul(
                        og[:, w:w + 17],
                        tiles[mc][:, i * NHF + nc4 * P:i * NHF + (nc4 + 1) * P],
                        vaug[:, mc, 2 * g2 + i, :],
                        start=(mc == 0), stop=(mc == MC - 1))

        def emit_norm(gi, og=None, nc0=0, ncn=NC4, suffix=""):
            """Normalize accumulators for nc4 range [nc0, nc0+ncn) held in
            `og` (windows start at (nc4-nc0)*68 cols)."""
            nh, g2 = groups[gi]
            st = state[gi]
            og = og if og is not None else st["og"]
            nw = ncn * 4
            ogv = og[:, 0:17 * nw].rearrange("p (w c) -> p w c", c=17)
            rec = recp.tile([P, 16], f32, tag="rec", name=f"rec_{gi}{suffix}")
            nc.vector.reciprocal_approx_fast(rec[:, 0:nw], ogv[:, :, 0])
            attn = state.get(("attn", nh))
            if attn is None:
                attn = atp.tile([P, NC4, 2, P], bf16, tag="attn",
                                name=f"attn_{nh}")
                state[("attn", nh)] = attn
            in0 = og[:, 0:17 * nw].rearrange(
                "p (nc a i c) -> p nc a i c", nc=ncn, a=2, i=2)[:, :, :, :, 1:17]
            in1 = rec[:, 0:nw].rearrange(
                "p (nc a i) -> p nc a i", nc=ncn, a=2)[:, :, :, :, None]
            out = attn[:, nc0:nc0 + ncn, :, 32 * g2:32 * g2 + 32].rearrange(
                "p nc a (i c) -> p nc a i c", i=2)
            nc.vector.tensor_mul(out, in0,
                                 in1.broadcast_to([P, ncn, 2, 2, 16]))

        def setup_tr(nh):
            # window idx 0-3 -> bank-0 cols, idx 4-7 -> bank-1 cols, so one
            # strided copy drains each 4-window bank
            if nh == NH - 1:
                tr = s4p.tile([P, N], f32, tag="s4", name="tr_final")
                t16 = tr[:].bitcast(bf16).rearrange("p (i c) -> p i c", c=P)
            else:
                tr = outp.tile([P, 4, P], f32, tag="outp", name=f"tr_{nh}")
                t16 = tr[:].bitcast(bf16).rearrange(
                    "p a (j c) -> p (a j) c", j=2)
            win = lambda idx: t16[:, (idx // 4) * (8 if nh == NH - 1 else 4)
                                  + idx % 4, :]
            state[("trwin", nh)] = win
            state[("trt16", nh)] = t16
            ctf = ctp.tile([P, 8, P], bf16, tag="ctf", name=f"ctf_{nh}")
            state[("ctf", nh)] = ctf

        def emit_transposes(nh):
            setup_tr(nh)
            emit_tr_sub(nh, range(NC4))

        def emit_tr_sub(nh, nc4s):
            attn = state[("attn", nh)]
            win = state[("trwin", nh)]
            for nc4 in nc4s:
                for a in range(2):
                    nc.tensor.transpose(win(nc4 * 2 + a),
                                        attn[:, nc4, a, :], ident)
            emit_ct_copies(nh, nc4s)

        def emit_ct_copies(nh, nc4s):
            # one batched copy per 4-window group (= per bank)
            t16 = state[("trt16", nh)]
            ctf = state[("ctf", nh)]
            cts = state.setdefault(("cts", nh), {})
            idxs = sorted(nc4 * 2 + a for nc4 in nc4s for a in range(2))
            base = 8 if nh == NH - 1 else 4
            for grp0 in (0, 4):
                grp = [i for i in idxs if grp0 <= i < grp0 + 4]
                if not grp:
                    continue
                lo, hi = min(grp), max(grp)
                src_ap = t16[:, (grp0 // 4) * base + lo % 4:
                             (grp0 // 4) * base + hi % 4 + 1, :]
                nc.vector.tensor_copy(ctf[:, lo:hi + 1, :], src_ap)
            for nc4 in nc4s:
                cts[nc4] = [ctf[:, nc4 * 2, :], ctf[:, nc4 * 2 + 1, :]]

        def emit_proj(nh, nc4s=range(NC4), part=0):
            cts = state[("cts", nh)]
            if part == 0 or ("op", nh) not in state:
                state[("op", nh)] = outp.tile([P, NC4, P], f32, tag="outp",
                                              name=f"op_{nh}")
                state[("ob", nh)] = osb.tile([P, NC4, P], f32, tag="osb",
                                             name=f"ob_{nh}")
            op = state[("op", nh)]
            ob = state[("ob", nh)]
            for nc4 in nc4s:
                sl = op[:, nc4, :]
                nc.tensor.matmul(sl, ones1[:], bor, start=True, stop=False)
                nc.tensor.matmul(sl, cts[nc4][0][:], wog,
                                 start=False, stop=False)
                nc.tensor.matmul(sl, cts[nc4][1][:], wog,
                                 start=False, stop=True)
            lo, hi = min(nc4s), max(nc4s) + 1
            if nh == NH - 1 and part == 0:
                nc.scalar.copy(ob[:, lo:hi, :], op[:, lo:hi, :])
            else:
                nc.vector.tensor_copy(ob[:, lo:hi, :], op[:, lo:hi, :])
            dst = out_d.ap().rearrange("(x c p) d -> x p c d", x=NH, c=NC4)[nh]
            nc.sync.dma_start(dst[:, lo:hi, :], ob[:, lo:hi, :])

        # accumulator emission order: eg-based accs first (their inputs are
        # ready one em-latency earlier at the group boundary)
        EG_ACCS = [a for a in range(16) if (a % 4) // 2 == 0]
        EM_ACCS = [a for a in range(16) if (a % 4) // 2 == 1]

        def emit_s4(gi, mc):
            nh, g2 = groups[gi]
            s4 = s4p.tile([P, N], f32, tag="s4", name=f"s4_{gi}_{mc}")
            t = "a" if g2 < 2 else "b"
            qT = qk_tiles["q" + t]
            kT = qk_tiles["k" + t]
            q0 = (2 * g2) % 4
            for i in range(2):
                qq = 32 * (q0 + i)
                nc.tensor.matmul(
                    s4[:, i * NHF:(i + 1) * NHF],
                    kT[qq:qq + 16, mc * P:(mc + 1) * P],
                    qT[qq:qq + 16, nh * NHF:(nh + 1) * NHF],
                    start=True, stop=True, tile_position=(qq, 0))
            return s4

        def emit_sf(gi, mc, i):
            """Score matmul for head-half i of chunk mc via the fxp bank."""
            nh, g2 = groups[gi]
            sf = fxp.tile([P, NHF], f32, tag="sf", name=f"sf_{gi}_{mc}_{i}")
            q0 = (2 * g2) % 4
            qq = 32 * (q0 + i)
            t = "a" if g2 < 2 else "b"
            nc.tensor.matmul(
                sf[:], qk_tiles["k" + t][qq:qq + 16, mc * P:(mc + 1) * P],
                qk_tiles["q" + t][qq:qq + 16, nh * NHF:(nh + 1) * NHF],
                start=True, stop=True, tile_position=(qq, 0))
            return sf

        def emit_exp(gi, mc, s4):
            eg = egp.tile([P, N], bf16, tag=f"eg{mc}", name=f"eg_{gi}_{mc}")
            nc.scalar.activation(eg[:], s4[:], Exp)
            return eg

        def emit_em(gi, mc, eg):
            nh, g2 = groups[gi]
            em = emp.tile([P, N], bf16, tag=f"em{mc}", name=f"em_{gi}_{mc}")
            msk = mask_sb[:, mc, nh, :]
            nc.vector.tensor_mul(
                em[:].rearrange("p (i f) -> p i f", i=2),
                eg[:].rearrange("p (i f) -> p i f", i=2),
                msk[:, None, :].broadcast_to([P, 2, NHF]))
            return em

        for gi in range(NG):
            nh, g2 = groups[gi]
            if gi > 0:
                state[gi - 1]["og"] = ogp.tile([P, 512], f32, tag="og",
                                               name=f"og_{gi - 1}")
            egs = [None] * MC
            ems = [None] * MC
            # fexp chunk (mc=0): half 0 scores + DVE bitcast exp
            eg0 = egp.tile([P, N], bf16, tag="eg0", name=f"eg_{gi}_0")
            sf0 = emit_sf(gi, 0, 0)
            nc.vector.tensor_scalar(eg0[:, 0:NHF].bitcast(u16), sf0[:],
                                    FE_A16, FE_B16,
                                    op0=Alu.mult, op1=Alu.add)
            s4_1 = emit_s4(gi, 1)
            egs[1] = emit_exp(gi, 1, s4_1)
            s4_2 = emit_s4(gi, 2)
            egs[2] = emit_exp(gi, 2, s4_2)
            sf1 = emit_sf(gi, 0, 1)
            nc.vector.tensor_scalar(eg0[:, NHF:N].bitcast(u16), sf1[:],
                                    FE_A16, FE_B16,
                                    op0=Alu.mult, op1=Alu.add)
            egs[0] = eg0
            ems[0] = emit_em(gi, 0, eg0)
            # first half of previous group's attn@V accumulators
            if gi > 0:
                emit_og_accs(gi - 1, EG_ACCS)
            ems[1] = emit_em(gi, 1, egs[1])
            ems[2] = emit_em(gi, 2, egs[2])

            if gi == 0:
                emit_b_projections()
                emit_vp()
            s4_3 = emit_s4(gi, 3)
            egs[3] = emit_exp(gi, 3, s4_3)
            ems[3] = emit_em(gi, 3, egs[3])
            if gi > 0:
                emit_og_accs(gi - 1, EM_ACCS)
                emit_norm(gi - 1)
            if gi == 5:
                emit_transposes(0)
            if gi == 7:
                emit_proj(0)
            for mc in range(4, MC):
                s4 = emit_s4(gi, mc)
                egs[mc] = emit_exp(gi, mc, s4)
                ems[mc] = emit_em(gi, mc, egs[mc])
                if gi == 0 and mc in (4, 6):
                    half = (mc - 4) // 2
                    vp4 = vaug_copies[half]
                    nc.vector.tensor_copy(
                        vaug[:, half * 4:half * 4 + 4, :, 1:17],
                        vp4.rearrange("p c (h d) -> p c h d", h=H))
            state[gi] = {"egs": egs, "ems": ems}

        # drain: pipelined across the two og banks (nc4 halves)
        gi = NG - 1
        og7a = ogp.tile([P, 512], f32, tag="og", name="og7a")
        og7b = ogp.tile([P, 512], f32, tag="og", name="og7b")
        # eg-based accumulators first: ready one em-latency earlier
        emit_og_accs(gi, [0, 1, 4, 5, 2, 3, 6, 7], og=og7a, wbase=0)
        emit_og_accs(gi, [8, 9, 12, 13, 10, 11, 14, 15], og=og7b, wbase=8)
        emit_norm(gi, og=og7a, nc0=0, ncn=2, suffix="a")
        setup_tr(NH - 1)
        emit_tr_sub(NH - 1, (0, 1))
        emit_norm(gi, og=og7b, nc0=2, ncn=2, suffix="b")
        emit_proj(NH - 1, nc4s=(0, 1), part=0)
        emit_tr_sub(NH - 1, (2, 3))
        emit_proj(NH - 1, nc4s=(2, 3), part=1)

    nc.compile()
    return nc


def _host_prep(query, key, value, adj_mask, Wq, bq, Wk, bk, Wv, bv, Wo, bo):
    """Per-core input maps (host-side layout transforms only)."""
    f32 = np.float32
    query = np.asarray(query, f32)
    key = np.asarray(key, f32)
    value = np.asarray(value, f32)
    Wq = np.asarray(Wq, f32); Wk = np.asarray(Wk, f32)
    Wv = np.asarray(Wv, f32); Wo = np.asarray(Wo, f32)
    bq = np.asarray(bq, f32); bk = np.asarray(bk, f32)
    bv = np.asarray(bv, f32); bo = np.asarray(bo, f32)
    adj = np.asarray(adj_mask)

    scale = 1.0 / np.sqrt(np.float32(DH))

    def pack_w(Wm):
        out = []
        for t in range(2):
            wt = np.zeros((P, P), f32)
            for j in range(4):
                h = 4 * t + j
                wt[:, 32 * j:32 * j + 16] = Wm[:, DH * h:DH * (h + 1)]
            out.append(wt)
        return out

    wqa, wqb = [w * scale for w in pack_w(Wq)]
    wka, wkb = pack_w(Wk)

    def pack_b(bvec, s):
        cols = []
        for t in range(2):
            col = np.zeros((P,), f32)
            for j in range(4):
                h = 4 * t + j
                col[32 * j:32 * j + 16] = bvec[DH * h:DH * (h + 1)] * s
            cols.append(col)
        return cols

    bqa, bqb = pack_b(bq, scale)
    bka, bkb = pack_b(bk, 1.0)

    cw = np.zeros((P, 4 * P), f32)
    cw[:, 0:P] = wqa; cw[:, P:2 * P] = wqb
    cw[:, 2 * P:3 * P] = wka; cw[:, 3 * P:4 * P] = wkb
    bias4 = np.stack([bqa, bqb, bka, bkb], axis=1).astype(f32)  # [P, 4]
    cw_b = np.concatenate([cw.astype(_BF16),
                           bias4.view(_BF16).reshape(P, 8)], axis=1)

    cbf = np.zeros((P, 3 * P), f32)
    cbf[:, 0:P] = Wv
    cbf[:, P:2 * P] = 0.5 * Wo
    cbf[:, 2 * P:3 * P] = np.eye(P, dtype=f32)
    cbf_b = cbf.astype(_BF16)

    br = np.concatenate([bv, bo]).reshape(1, 2 * P)

    maskT = adj.T.astype(f32)  # [m, n]
    maskL = maskT.reshape(MC, P, NH, NHF).transpose(1, 0, 2, 3).reshape(P, -1)

    shared = {
        "cw": cw_b,
        "cbf": cbf_b,
        "br": br.astype(_BF16),
        "maskL": maskL.astype(_BF16),
    }
    in_maps = []
    for b in range(B):
        m = dict(shared)
        m["xqT"] = np.ascontiguousarray(query[b].T).astype(_BF16)
        m["xkT"] = np.ascontiguousarray(key[b].T).astype(_BF16)
        m["xvT"] = np.ascontiguousarray(value[b].T).astype(_BF16)
        in_maps.append(m)
    return in_maps


def kernel(**inputs):
    if "nc" not in _CACHE:
        _CACHE["nc"] = _build_nc()
    nc = _CACHE["nc"]

    from concourse.bass_utils import run_bass_kernel_spmd

    in_maps = _host_prep(**inputs)
    res = run_bass_kernel_spmd(nc, in_maps, core_ids=list(range(NCORES)))
    out = np.stack([res.results[c]["out"] for c in range(NCORES)], axis=0)
    return out.astype(np.float32)
